# revision 1
# baseline (speedup 1.0000x reference)
"""Trainium2 Bass kernel for nn_DSA2Attention (MLA-latent sparse sliding-window attention).

Strategy (tensor-parallel over heads, 8 cores, 2 heads/core):
  host:  fold Wc into Wk/Wv (k = x @ (Wc@Wk) etc), permute q/k head-dims so rope
         pairs become [x1(0:64); x2(64:128)], precompute rope cos/sin tables in
         [d', t] layout, sliding-window triangle masks, identity.
  device (per core, SPMD — identical program, different weight slices):
    phase 1: qT[d,t], kT[d,t] (feature-major) and v[t,d] via PE matmuls from
             xT chunks; rope on DVE; block-mean kbT via segmented reduce.
    phase 2: per query-tile of 128: S = qT.T@kT over <=5 key tiles (sliding
             window) + bsc = qT.T@kbT in one PSUM tile; top-16-of-32 block
             boost via max8/match_replace; boost broadcast-add + triangle
             masks; exp (no row-max needed: scores bounded) with accumulated
             row sums; P.T via PE "transpose" against diag(1/r) (folds the
             softmax normalization in for free); AV accumulate attnT[d,q];
             out-projection psum -> DMA straight to DRAM.
  host:  sum the 8 partial projections (row-parallel Wo) + bias.

Numerics: matmul operands in bf16 (fp32 PSUM accumulation), softmax chain in
fp32, output partials bf16 summed in fp64 on host. Measured relative error vs
the fp32 reference: ~3.7e-3 (absmax-relative). Masks use -1e30/SCALE additive.
Set MM_DT=f32 for an fp32-exact (~2.3x slower) variant.
"""
import os
import numpy as np

import concourse.bacc as bacc
import concourse.bass as bass
import concourse.mybir as mybir
import concourse.tile as tile
from concourse.bass_utils import run_bass_kernel_spmd

B, T, D = 1, 2048, 2048
NH, NKV, HD = 16, 4, 128
KVC = 512
WIN = 512
BS = 64
NSEL = 16
SCALE = HD ** -0.5
NB = T // BS          # 32
NCORE = 8
HPC = NH // NCORE     # heads per core = 2

KT = T // 128         # 16 k-tiles
NCH = 4               # phase-1 t-chunks
CH = T // NCH         # 512
QT = T // 128         # 16 query tiles
NEG = -1e30
MASKV = -1e30 / SCALE

F32 = mybir.dt.float32
R32 = mybir.dt.float32r
AF = mybir.ActivationFunctionType
OP = mybir.AluOpType

# dtype for every tensor that feeds a matmul. bf16: 1 cyc/row + fast weight
# load (fp32: 2 cyc/row, exact; fp32r crashes the exec unit on this toolchain).
MM_DT = os.environ.get("MM_DT", "bf16")
MMDT = {"bf16": mybir.dt.bfloat16, "f32": F32, "f32r": R32}[MM_DT]
NPMM = None  # numpy dtype for host-side arrays feeding matmuls; set below


def _mr(ap):
    return ap

_cache = {}


def build_nc(trace_label=""):
    nc = bacc.Bacc("TRN2", target_bir_lowering=False, debug=False, num_devices=NCORE)

    xT_d = nc.dram_tensor("xT", [D, T], MMDT, kind="ExternalInput").ap()
    wq_d = nc.dram_tensor("wq", [D, HPC * HD], MMDT, kind="ExternalInput").ap()
    wck_d = nc.dram_tensor("wck", [D, HD], MMDT, kind="ExternalInput").ap()
    wcv_d = nc.dram_tensor("wcv", [D, HD], MMDT, kind="ExternalInput").ap()
    wo_d = nc.dram_tensor("wo", [HPC * HD, D], MMDT, kind="ExternalInput").ap()
    cos2_d = nc.dram_tensor("cos2", [HD, T], F32, kind="ExternalInput").ap()
    sin2_d = nc.dram_tensor("sin2", [HD, T], F32, kind="ExternalInput").ap()
    tri_lo_d = nc.dram_tensor("tri_lo", [128, 128], F32, kind="ExternalInput").ap()
    tri_hi_d = nc.dram_tensor("tri_hi", [128, 128], F32, kind="ExternalInput").ap()
    eye_d = nc.dram_tensor("eye", [128, 128], F32, kind="ExternalInput").ap()
    jt_d = nc.dram_tensor("jt", [128, 128], MMDT, kind="ExternalInput").ap()
    bias3_d = nc.dram_tensor("bias3", [HD, 3], F32, kind="ExternalInput").ap()
    out_d = nc.dram_tensor("out", [T, D], mybir.dt.bfloat16,
                           kind="ExternalOutput").ap()

    with tile.TileContext(nc) as tc:
        with tc.tile_pool(name="persist", bufs=1) as pp:
            wq_sb = pp.tile([128, KT * HPC * HD], MMDT, tag="wq")
            wck_sb = pp.tile([128, KT * HD], MMDT, tag="wck")
            wcv_sb = pp.tile([128, KT * HD], MMDT, tag="wcv")
            wo_sb = pp.tile([128, HPC * D], MMDT, tag="wo")
            cos2_sb = pp.tile([128, T], F32, tag="cos2")
            sin2_sb = pp.tile([128, T], F32, tag="sin2")
            tri_lo = pp.tile([128, 128], F32, tag="tri_lo")
            tri_hi = pp.tile([128, 128], F32, tag="tri_hi")
            eye_sb = pp.tile([128, 128], F32, tag="eye")
            jt_sb = pp.tile([128, 128], MMDT, tag="jt")
            eyeb = pp.tile([128, 128], MMDT, tag="eyeb")
            bias3 = pp.tile([128, 3], F32, tag="bias3")
            qT_sb = pp.tile([128, HPC * T], MMDT, tag="qT")
            kT_sb = pp.tile([128, T], MMDT, tag="kT")
            v_sb = pp.tile([128, KT * HD], MMDT, tag="v")
            kbT = pp.tile([128, NB], MMDT, tag="kbT")

            nc.sync.dma_start(wq_sb[:, :HPC * HD], wq_d[0:128, :])
            nc.sync.dma_start(wck_sb[:, :HD], wck_d[0:128, :])
            nc.sync.dma_start(wcv_sb[:, :HD], wcv_d[0:128, :])
            nc.sync.dma_start(
                wq_sb[:, HPC * HD:].rearrange("p (k d) -> p k d", k=KT - 1),
                wq_d[128:, :].rearrange("(k p) d -> p k d", p=128))
            nc.sync.dma_start(
                wck_sb[:, HD:].rearrange("p (k d) -> p k d", k=KT - 1),
                wck_d[128:, :].rearrange("(k p) d -> p k d", p=128))
            nc.sync.dma_start(
                wcv_sb[:, HD:].rearrange("p (k d) -> p k d", k=KT - 1),
                wcv_d[128:, :].rearrange("(k p) d -> p k d", p=128))

            # ---------- phase 1: qT, kT, vT ----------
            # xT stays resident in SBUF (bf16: 64KB/partition).
            xt_sb = pp.tile([128, KT * T], MMDT, tag="xt")
            vT_sb = pp.tile([128, T], MMDT, tag="vT")
            boost_all = pp.tile([128, QT * HPC * NB], F32, tag="boost_all")
            for kt in range(KT):      # chunk-0 x first so the PE starts early
                nc.sync.dma_start(
                    xt_sb[:, kt * T:kt * T + CH],
                    xT_d[kt * 128:(kt + 1) * 128, 0:CH])
            nc.sync.dma_start(bias3[:], bias3_d)
            nc.sync.dma_start(cos2_sb[:], cos2_d)
            nc.sync.dma_start(sin2_sb[:], sin2_d)
            nc.sync.dma_start(jt_sb[:], jt_d)
            nc.sync.dma_start(tri_lo[:], tri_lo_d)
            nc.sync.dma_start(tri_hi[:], tri_hi_d)
            nc.sync.dma_start(eye_sb[:], eye_d)
            nc.vector.tensor_copy(eyeb[:], eye_sb[:])
            for ch in range(1, NCH):
                for kt in range(KT):
                    nc.sync.dma_start(
                        xt_sb[:, kt * T + ch * CH:kt * T + (ch + 1) * CH],
                        xT_d[kt * 128:(kt + 1) * 128, ch * CH:(ch + 1) * CH])
            nc.sync.dma_start(
                wo_sb[:].rearrange("p (h e) -> p h e", h=HPC),
                wo_d.rearrange("(h p) e -> p h e", p=128))
            with tc.tile_pool(name="rs", bufs=3) as rsp, \
                 tc.tile_pool(name="psA", bufs=8, space="PSUM") as psA:
                def p1_mms(ch):
                    qd = [psA.tile([128, CH], F32, tag="qkT", name=f"qd{ch}_{_h}")
                          for _h in range(HPC)]
                    kTp = psA.tile([128, CH], F32, tag="qkT", name=f"kTp{ch}")
                    vTp = psA.tile([128, CH], F32, tag="qkT", name=f"vTp{ch}")
                    for kt in range(KT):
                        xt = xt_sb[:, kt * T + ch * CH:kt * T + (ch + 1) * CH]
                        st = dict(start=(kt == 0), stop=(kt == KT - 1))
                        for h in range(HPC):
                            nc.tensor.matmul(
                                qd[h][:],
                                lhsT=wq_sb[:, kt * HPC * HD + h * HD:
                                           kt * HPC * HD + (h + 1) * HD],
                                rhs=xt, **st)
                        nc.tensor.matmul(
                            kTp[:], lhsT=wck_sb[:, kt * HD:(kt + 1) * HD],
                            rhs=xt, **st)
                        nc.tensor.matmul(
                            vTp[:], lhsT=wcv_sb[:, kt * HD:(kt + 1) * HD],
                            rhs=xt, **st)
                    return qd, kTp, vTp

                def p1_rope(ch, qd, kTp, vTp):
                    cs = slice(ch * CH, (ch + 1) * CH)
                    # rope + bias: dst = (ps+b)*cos2 + J @ ((ps+b)*sin2)
                    # (J = [[0,-I64],[I64,0]] does the half-swap on the PE;
                    #  cross-partition DVE ops are illegal on HW)
                    for ti, (ps, dst) in enumerate(
                            [(qd[0], qT_sb[:, 0 * T + ch * CH:0 * T + (ch + 1) * CH]),
                             (qd[1], qT_sb[:, 1 * T + ch * CH:1 * T + (ch + 1) * CH]),
                             (kTp, kT_sb[:, cs])]):
                        U = rsp.tile([128, CH], F32, tag="U", name=f"U{ch}_{ti}")
                        Wt = rsp.tile([128, CH], MMDT, tag="W", name=f"Wt{ch}_{ti}")
                        b = bias3[:, ti:ti + 1]
                        nc.vector.scalar_tensor_tensor(
                            U[:], ps[:], b, cos2_sb[:, cs], op0=OP.add, op1=OP.mult)
                        nc.vector.scalar_tensor_tensor(
                            Wt[:], ps[:], b, sin2_sb[:, cs], op0=OP.add, op1=OP.mult)
                        rp = psA.tile([128, CH], F32, tag="qkT", name=f"rp{ch}_{ti}")
                        nc.tensor.matmul(rp[:], lhsT=jt_sb[:], rhs=Wt[:],
                                         start=True, stop=True)
                        nc.vector.tensor_add(dst, rp[:], U[:])
                    nc.any.tensor_copy(vT_sb[:, cs], vTp[:])

                prev = None
                for ch in range(NCH):
                    cur = p1_mms(ch)
                    if prev is not None:
                        p1_rope(ch - 1, *prev)
                    prev = cur
                p1_rope(NCH - 1, *prev)

                # v[t, d] from vT[d, t] via one xbar transpose (bf16)
                nc.sync.dma_start_transpose(
                    v_sb[:].rearrange("p (k f) -> p k f", k=KT), vT_sb[:])

                # block means of roped kT: [128, T] -> [128, NB], 1/BS scale
                with nc.allow_low_precision(reason="bf16 block-mean output"):
                    nc.vector.reduce_sum(
                        kbT[:, :, None],
                        kT_sb[:].rearrange("p (b i) -> p b i", b=NB),
                        axis=mybir.AxisListType.X)
                nc.vector.tensor_scalar_mul(kbT[:], kbT[:], 1.0 / BS)
            # bulk block scores + top-16 boost for every (h, qt) --
            # keeps the 5-op DVE chain off phase 2's critical path
            if True:
                with tc.tile_pool(name="psB", bufs=4, space="PSUM") as psB, \
                     tc.tile_pool(name="pTk", bufs=6) as pTk:
                    for h in range(HPC):
                        for qt in range(QT):
                            qTh = qT_sb[:, h * T + qt * 128:h * T + (qt + 1) * 128]
                            bsc = psB.tile([128, NB], F32, tag="bsc",
                                           name=f"bsc{qt}_{h}")
                            nc.tensor.matmul(bsc[:], lhsT=qTh, rhs=kbT[:],
                                             start=True, stop=True)
                            z = pTk.tile([128, NB], F32, tag="z", name=f"z{qt}_{h}")
                            m8 = pTk.tile([128, 8], F32, tag="m8",
                                          name=f"m8_{qt}_{h}")
                            nc.scalar.copy(z[:], bsc[:])
                            for _ in range(NSEL // 8):
                                nc.vector.max(out=m8[:], in_=z[:])
                                nc.vector.match_replace(
                                    out=z[:], in_to_replace=m8[:], in_values=z[:],
                                    imm_value=NEG)
                            bo_sl = boost_all[:, (h * QT + qt) * NB:
                                              (h * QT + qt + 1) * NB]
                            nc.vector.scalar_tensor_tensor(
                                bo_sl, z[:], NEG, bsc[:],
                                op0=OP.is_le, op1=OP.mult)

            # ---------- phase 2: attention + projection ----------
            # Stage A(qt): S matmuls -> fused boost-add copy to SBUF (frees
            # PSUM) -> masks -> exp -> normalize to bf16.
            # Stage B(qt): xbar-transpose P -> AV -> projection -> DMA out.
            # Emitted software-pipelined: A(qt) then B(qt-1), so PE always has
            # adjacent work and HAM stays warm.
            with tc.tile_pool(name="psS", bufs=2, space="PSUM") as psS, \
                 tc.tile_pool(name="psAcc", bufs=2, space="PSUM") as psAcc, \
                 tc.tile_pool(name="psPT", bufs=2, space="PSUM") as psPT, \
                 tc.tile_pool(name="pSb", bufs=6) as pSb, \
                 tc.tile_pool(name="pPn", bufs=3) as pPn, \
                 tc.tile_pool(name="pPT", bufs=2) as pPT, \
                 tc.tile_pool(name="pA", bufs=4) as pA, \
                 tc.tile_pool(name="pOut", bufs=2) as pOut, \
                 tc.tile_pool(name="pSm", bufs=8) as pSm:
                pn_tiles = {}

                def stage_a(qt):
                    nk = min(qt, 4) + 1
                    kt0 = qt + 1 - nk
                    Pn = pPn.tile([128, 2 * 640], MMDT, tag="Pn", name=f"Pn{qt}")
                    pn_tiles[qt] = Pn
                    for h in range(HPC):
                        qTh = qT_sb[:, h * T + qt * 128:h * T + (qt + 1) * 128]
                        S = psS.tile([128, 640], F32, tag="S", name=f"S{qt}_{h}")
                        o = 0
                        while o < nk * 128:          # <=512-wide, bank-aligned
                            w = min(512, nk * 128 - o)
                            nc.tensor.matmul(
                                S[:, o:o + w], lhsT=qTh,
                                rhs=kT_sb[:, kt0 * 128 + o:kt0 * 128 + o + w],
                                start=True, stop=True)
                            o += w
                        # psum->sbuf copy (frees PSUM regardless of top-k
                        # progress), then boost broadcast-add on idle GpSimd
                        Ssb = pSb.tile([128, 640], F32, tag="Ssb", name=f"Ssb{qt}_{h}")
                        nc.vector.tensor_copy(Ssb[:, :nk * 128], S[:, :nk * 128])
                        bo_sl = boost_all[:, (h * QT + qt) * NB + 2 * kt0:
                                          (h * QT + qt) * NB + 2 * (qt + 1)]
                        bv = bo_sl[:, :, None].to_broadcast([128, 2 * nk, BS])
                        Sv = Ssb[:, :nk * 128].rearrange("p (b i) -> p b i", i=BS)
                        nc.gpsimd.tensor_tensor(out=Sv, in0=Sv, in1=bv, op=OP.add)
                        # sliding-window triangle masks (in SBUF)
                        if qt >= 4:
                            nc.vector.tensor_add(Ssb[:, 0:128], Ssb[:, 0:128],
                                                 tri_hi[:])
                        nc.vector.tensor_add(
                            Ssb[:, (nk - 1) * 128:nk * 128],
                            Ssb[:, (nk - 1) * 128:nk * 128], tri_lo[:])
                        # exp (scores bounded; no row-max) + row sums
                        Praw = pSb.tile([128, 640], F32, tag="P", name=f"P{qt}_{h}")
                        r = pSm.tile([128, 1], F32, tag="r", name=f"r{qt}_{h}")
                        rinv = pSm.tile([128, 1], F32, tag="rinv",
                                        name=f"rinv{qt}_{h}")
                        nc.scalar.activation(
                            Praw[:, :nk * 128], Ssb[:, :nk * 128], AF.Exp,
                            scale=float(SCALE), accum_out=r[:])
                        nc.vector.reciprocal(rinv[:], r[:])
                        # normalize + bf16 cast on ACT
                        nc.scalar.activation(
                            Pn[:, h * nk * 128:(h + 1) * nk * 128],
                            Praw[:, :nk * 128], AF.Copy, scale=rinv[:])

                def stage_b(qt):
                    nk = min(qt, 4) + 1
                    kt0 = qt + 1 - nk
                    Pn = pn_tiles.pop(qt)
                    PnT = pPT.tile([128, 2 * 640], MMDT, tag="PnT", name=f"PnT{qt}")
                    for hj in range(2 * nk):
                        ptp = psPT.tile([128, 128], MMDT, tag="ptp",
                                        name=f"ptp{qt}_{hj}")
                        nc.tensor.transpose(
                            ptp[:], Pn[:, hj * 128:(hj + 1) * 128], eyeb[:])
                        nc.any.tensor_copy(
                            PnT[:, hj * 128:(hj + 1) * 128], ptp[:])
                    a_sb = []
                    for h in range(HPC):
                        attnT = psAcc.tile([128, 128], F32, tag="acc",
                                           name=f"attnT{qt}_{h}")
                        for j in range(nk):
                            nc.tensor.matmul(
                                attnT[:],
                                lhsT=v_sb[:, (kt0 + j) * 128:(kt0 + j + 1) * 128],
                                rhs=PnT[:, (h * nk + j) * 128:(h * nk + j + 1) * 128],
                                start=(j == 0), stop=(j == nk - 1))
                        at = pA.tile([128, 128], MMDT, tag="at", name=f"at{qt}_{h}")
                        nc.any.tensor_copy(at[:], attnT[:])
                        a_sb.append(at[:])
                    ot = pOut.tile([128, D], mybir.dt.bfloat16, tag="ot",
                                   name=f"ot{qt}")
                    for e in range(4):
                        pr = psAcc.tile([128, 512], F32, tag="acc",
                                        name=f"pr{qt}_{e}")
                        for h in range(HPC):
                            nc.tensor.matmul(
                                pr[:], lhsT=a_sb[h],
                                rhs=wo_sb[:, h * D + e * 512:h * D + (e + 1) * 512],
                                start=(h == 0), stop=(h == HPC - 1))
                        nc.any.tensor_copy(ot[:, e * 512:(e + 1) * 512], pr[:])
                    nc.sync.dma_start(out_d[qt * 128:(qt + 1) * 128, :], ot[:])

                for qt in range(QT + 1):
                    if qt < QT:
                        stage_a(qt)
                    if qt >= 1:
                        stage_b(qt - 1)
    nc.compile()
    return nc


# ---------------------------------------------------------------- host side

def _np_mm(a):
    """Convert a host array to the matmul wire dtype."""
    import concourse.mybir as _mb
    return np.ascontiguousarray(a).astype(_mb.dt.np(MMDT))


def _host_prep(x, Wc, bc, Wk, bk, Wv, bv, Wq, bq, Wo, bo, loop_idx):
    f = np.float32
    x = np.asarray(x, f).reshape(T, D)
    Wc, Wk, Wv, Wq, Wo = (np.asarray(a, f) for a in (Wc, Wk, Wv, Wq, Wo))
    bc, bk, bv, bq, bo = (np.asarray(a, f) for a in (bc, bk, bv, bq, bo))
    li = int(np.asarray(loop_idx))

    xT = np.ascontiguousarray(x.T)
    Wck = Wc @ Wk
    bck = bc @ Wk + bk
    Wcv = Wc @ Wv
    bcv = bc @ Wv + bv

    pos = (np.arange(T) + li * T).astype(np.float64)
    inv = 1.0 / (10000.0 ** (np.arange(0, HD, 2).astype(np.float64) / HD))
    ang = pos[:, None] * inv[None, :]                       # [T, 64]
    cos = np.cos(ang).astype(f)
    sin = np.sin(ang).astype(f)
    cos2 = np.ascontiguousarray(np.concatenate([cos, cos], axis=1).T)  # [128, T]
    sin2 = np.ascontiguousarray(np.concatenate([sin, sin], axis=1).T)

    perm = np.concatenate([np.arange(0, HD, 2), np.arange(1, HD, 2)])

    a = np.arange(128)
    tri_lo = np.where(a[None, :] <= a[:, None], 0.0, MASKV).astype(f)  # causal
    tri_hi = np.where(a[None, :] >= a[:, None], 0.0, MASKV).astype(f)
    eye = np.eye(128, dtype=f)
    J = np.zeros((128, 128), f)
    J[np.arange(64), np.arange(64) + 64] = -1.0
    J[np.arange(64) + 64, np.arange(64)] = 1.0
    jt = np.ascontiguousarray(J.T)

    in_maps = []
    bo_eff = bo.copy()
    for c in range(NCORE):
        h0 = HPC * c
        g = h0 // (NH // NKV)
        Wq_c = Wq[:, h0 * HD:(h0 + HPC) * HD].reshape(D, HPC, HD)[:, :, perm]
        Wq_c = np.ascontiguousarray(Wq_c.reshape(D, HPC * HD))
        bq_c = bq[h0 * HD:(h0 + HPC) * HD].reshape(HPC, HD)[:, perm]
        Wck_c = np.ascontiguousarray(Wck[:, g * HD:(g + 1) * HD][:, perm])
        bck_c = bck[g * HD:(g + 1) * HD][perm]
        Wcv_c = np.ascontiguousarray(Wcv[:, g * HD:(g + 1) * HD])
        bcv_c = bcv[g * HD:(g + 1) * HD]
        Wo_c = np.ascontiguousarray(Wo[h0 * HD:(h0 + HPC) * HD, :])
        # v-bias folded through softmax (rows sum to 1): + bcv @ Wo_head, per head
        for hh in range(HPC):
            bo_eff = bo_eff + bcv_c @ Wo_c[hh * HD:(hh + 1) * HD]
        bias3 = np.stack([bq_c[0], bq_c[1], bck_c], axis=1).astype(f)  # [128, 3]
        in_maps.append({
            "xT": _np_mm(xT), "wq": _np_mm(Wq_c), "wck": _np_mm(Wck_c),
            "wcv": _np_mm(Wcv_c), "wo": _np_mm(Wo_c),
            "cos2": cos2, "sin2": sin2, "tri_lo": tri_lo, "tri_hi": tri_hi,
            "eye": eye, "jt": _np_mm(jt), "bias3": bias3,
        })
    return in_maps, bo_eff


def _maybe_install_ntff_hook():
    """This axon image lacks antenv.axon_hooks; synthesize it so
    run_bass_kernel_spmd(trace=True) can capture NTFFs. Best-effort."""
    try:
        import sys
        import types
        import antenv
        if getattr(antenv, "axon_hooks", None) is not None:
            return
        from trn_agent_boot.trn_boot import _ntff_profile_via_ctypes
        hook = _ntff_profile_via_ctypes("/opt/axon/libaxon_pjrt.so")
        mod = types.ModuleType("antenv.axon_hooks")
        mod._hook = hook
        mod.get_axon_ntff_profile_hook = lambda: mod._hook
        mod.set_axon_ntff_profile_hook = lambda h: setattr(mod, "_hook", h)
        sys.modules["antenv.axon_hooks"] = mod
        antenv.axon_hooks = mod
    except Exception as e:  # profiling is optional
        print(f"ntff hook install failed: {e}")


def kernel(**inputs) -> np.ndarray:
    in_maps, bo_eff = _host_prep(**inputs)
    if "nc" not in _cache:
        _cache["nc"] = build_nc()
    trace = bool(int(os.environ.get("KERNEL_TRACE", "0")))
    if trace:
        _maybe_install_ntff_hook()
    res = run_bass_kernel_spmd(
        _cache["nc"], in_maps, core_ids=list(range(NCORE)),
        trace=trace)
    if trace:
        _cache["last_results"] = res
    out = np.zeros((T, D), np.float64)
    for r in res.results:
        out += r["out"].astype(np.float64)
    out = (out + bo_eff.astype(np.float64)).astype(np.float32)
    return out.reshape(B, T, D)



# revision 6
# speedup vs baseline: 1.0103x; 1.0103x over previous
"""Trainium2 Bass kernel for nn_DSA2Attention (MLA-latent sparse sliding-window attention).

Strategy (tensor-parallel over heads, 8 cores, 2 heads/core):
  host:  fold Wc into Wk/Wv (k = x @ (Wc@Wk) etc), permute q/k head-dims so rope
         pairs become [x1(0:64); x2(64:128)], precompute rope cos/sin tables in
         [d', t] layout, sliding-window triangle masks, identity.
  device (per core, SPMD — identical program, different weight slices):
    phase 1: qT[d,t], kT[d,t] (feature-major) and v[t,d] via PE matmuls from
             xT chunks; rope on DVE; block-mean kbT via segmented reduce.
    phase 2: per query-tile of 128: S = qT.T@kT over <=5 key tiles (sliding
             window) + bsc = qT.T@kbT in one PSUM tile; top-16-of-32 block
             boost via max8/match_replace; boost broadcast-add + triangle
             masks; exp (no row-max needed: scores bounded) with accumulated
             row sums; P.T via PE "transpose" against diag(1/r) (folds the
             softmax normalization in for free); AV accumulate attnT[d,q];
             out-projection psum -> DMA straight to DRAM.
  host:  sum the 8 partial projections (row-parallel Wo) + bias.

Numerics: matmul operands in bf16 (fp32 PSUM accumulation), softmax chain in
fp32, output partials bf16 summed in fp64 on host. Measured relative error vs
the fp32 reference: ~3.7e-3 (absmax-relative). Masks use -1e30/SCALE additive.
Set MM_DT=f32 for an fp32-exact (~2.3x slower) variant.
"""
import os
import numpy as np

import concourse.bacc as bacc
import concourse.bass as bass
import concourse.mybir as mybir
import concourse.tile as tile
from concourse.bass_utils import run_bass_kernel_spmd

B, T, D = 1, 2048, 2048
NH, NKV, HD = 16, 4, 128
KVC = 512
WIN = 512
BS = 64
NSEL = 16
SCALE = HD ** -0.5
NB = T // BS          # 32
NCORE = 8
HPC = NH // NCORE     # heads per core = 2

KT = T // 128         # 16 k-tiles
NCH = 4               # phase-1 t-chunks
CH = T // NCH         # 512
QT = T // 128         # 16 query tiles
NEG = -1e30
MASKV = -1e30 / SCALE

F32 = mybir.dt.float32
R32 = mybir.dt.float32r
AF = mybir.ActivationFunctionType
OP = mybir.AluOpType

# dtype for every tensor that feeds a matmul. bf16: 1 cyc/row + fast weight
# load (fp32: 2 cyc/row, exact; fp32r crashes the exec unit on this toolchain).
MM_DT = os.environ.get("MM_DT", "bf16")
MMDT = {"bf16": mybir.dt.bfloat16, "f32": F32, "f32r": R32}[MM_DT]
NPMM = None  # numpy dtype for host-side arrays feeding matmuls; set below


def _mr(ap):
    return ap

_cache = {}


def build_nc(trace_label=""):
    nc = bacc.Bacc("TRN2", target_bir_lowering=False, debug=False, num_devices=NCORE)

    # Host-prepacked to the exact SBUF layouts: every DMA row is contiguous
    # DRAM (few large descriptors instead of thousands of 512B ones).
    xT_d = nc.dram_tensor("xT", [128, NCH * KT * CH], MMDT,
                          kind="ExternalInput").ap()
    wq_d = nc.dram_tensor("wq", [128, KT * HPC * HD], MMDT,
                          kind="ExternalInput").ap()
    wck_d = nc.dram_tensor("wck", [128, KT * HD], MMDT,
                           kind="ExternalInput").ap()
    wcv_d = nc.dram_tensor("wcv", [128, KT * HD], MMDT,
                           kind="ExternalInput").ap()
    wo_d = nc.dram_tensor("wo", [128, HPC * D], MMDT, kind="ExternalInput").ap()
    cos2_d = nc.dram_tensor("cos2", [HD, T], F32, kind="ExternalInput").ap()
    sin2_d = nc.dram_tensor("sin2", [HD, T], F32, kind="ExternalInput").ap()
    tri_lo_d = nc.dram_tensor("tri_lo", [128, 128], F32, kind="ExternalInput").ap()
    tri_hi_d = nc.dram_tensor("tri_hi", [128, 128], F32, kind="ExternalInput").ap()
    eye_d = nc.dram_tensor("eye", [128, 128], F32, kind="ExternalInput").ap()
    jt_d = nc.dram_tensor("jt", [128, 128], MMDT, kind="ExternalInput").ap()
    bias3_d = nc.dram_tensor("bias3", [HD, 3], F32, kind="ExternalInput").ap()
    out_d = nc.dram_tensor("out", [T, D], mybir.dt.bfloat16,
                           kind="ExternalOutput").ap()

    with tile.TileContext(nc) as tc:
        with tc.tile_pool(name="persist", bufs=1) as pp:
            wq_sb = pp.tile([128, KT * HPC * HD], MMDT, tag="wq")
            wck_sb = pp.tile([128, KT * HD], MMDT, tag="wck")
            wcv_sb = pp.tile([128, KT * HD], MMDT, tag="wcv")
            wo_sb = pp.tile([128, HPC * D], MMDT, tag="wo")
            cos2_sb = pp.tile([128, T], F32, tag="cos2")
            sin2_sb = pp.tile([128, T], F32, tag="sin2")
            tri_lo = pp.tile([128, 128], F32, tag="tri_lo")
            tri_hi = pp.tile([128, 128], F32, tag="tri_hi")
            eye_sb = pp.tile([128, 128], F32, tag="eye")
            jt_sb = pp.tile([128, 128], MMDT, tag="jt")
            eyeb = pp.tile([128, 128], MMDT, tag="eyeb")
            bias3 = pp.tile([128, 3], F32, tag="bias3")
            qT_sb = pp.tile([128, HPC * T], MMDT, tag="qT")
            kT_sb = pp.tile([128, T], MMDT, tag="kT")
            v_sb = pp.tile([128, KT * HD], MMDT, tag="v")
            kbT = pp.tile([128, NB], MMDT, tag="kbT")

            # PE warmup: trip the HAM clock-gate to 8/8 while the initial
            # DMAs are in flight so real matmuls start at 2.4 GHz.
            warm_sb = pp.tile([128, 512], MMDT, tag="warm")
            nc.vector.memset(warm_sb[:], 0.0)
            with tc.tile_pool(name="psW", bufs=2, space="PSUM") as psW:
                for i in range(12):
                    wps = psW.tile([128, 512], F32, tag="w", name=f"warm{i}")
                    nc.tensor.matmul(wps[:], lhsT=warm_sb[:, :128],
                                     rhs=warm_sb[:], start=True, stop=True)
                    if i == 11:
                        nc.vector.tensor_copy(warm_sb[:, :512], wps[:])

            # weight DMAs split into col-chunks so they spread across queues
            for i in range(4):
                w = KT * HPC * HD // 4
                nc.sync.dma_start(wq_sb[:, i * w:(i + 1) * w],
                                  wq_d[:, i * w:(i + 1) * w])
            for i in range(2):
                w = KT * HD // 2
                nc.sync.dma_start(wck_sb[:, i * w:(i + 1) * w],
                                  wck_d[:, i * w:(i + 1) * w])
                nc.sync.dma_start(wcv_sb[:, i * w:(i + 1) * w],
                                  wcv_d[:, i * w:(i + 1) * w])

            # ---------- phase 1: qT, kT, vT ----------
            # xT stays resident in SBUF (bf16: 64KB/partition), ch-major.
            xt_sb = pp.tile([128, NCH * KT * CH], MMDT, tag="xt")
            vT_sb = pp.tile([128, T], MMDT, tag="vT")
            boost_all = pp.tile([128, QT * HPC * NB], F32, tag="boost_all")
            nc.sync.dma_start(xt_sb[:, 0:KT * CH], xT_d[:, 0:KT * CH])
            nc.sync.dma_start(bias3[:], bias3_d)
            nc.sync.dma_start(cos2_sb[:], cos2_d)
            nc.sync.dma_start(sin2_sb[:], sin2_d)
            nc.sync.dma_start(jt_sb[:], jt_d)
            nc.sync.dma_start(tri_lo[:], tri_lo_d)
            nc.sync.dma_start(tri_hi[:], tri_hi_d)
            nc.sync.dma_start(eye_sb[:], eye_d)
            nc.vector.tensor_copy(eyeb[:], eye_sb[:])
            for ch in range(1, NCH):
                nc.sync.dma_start(
                    xt_sb[:, ch * KT * CH:(ch + 1) * KT * CH],
                    xT_d[:, ch * KT * CH:(ch + 1) * KT * CH])
            for i in range(2):
                w = HPC * D // 2
                nc.sync.dma_start(wo_sb[:, i * w:(i + 1) * w],
                                  wo_d[:, i * w:(i + 1) * w])
            with tc.tile_pool(name="rs", bufs=3) as rsp, \
                 tc.tile_pool(name="psA", bufs=8, space="PSUM") as psA:
                def p1_mms(ch):
                    qd = [psA.tile([128, CH], F32, tag="qkT", name=f"qd{ch}_{_h}")
                          for _h in range(HPC)]
                    kTp = psA.tile([128, CH], F32, tag="qkT", name=f"kTp{ch}")
                    vTp = psA.tile([128, CH], F32, tag="qkT", name=f"vTp{ch}")
                    for kt in range(KT):
                        xt = xt_sb[:, (ch * KT + kt) * CH:(ch * KT + kt + 1) * CH]
                        st = dict(start=(kt == 0), stop=(kt == KT - 1))
                        for h in range(HPC):
                            nc.tensor.matmul(
                                qd[h][:],
                                lhsT=wq_sb[:, kt * HPC * HD + h * HD:
                                           kt * HPC * HD + (h + 1) * HD],
                                rhs=xt, **st)
                        nc.tensor.matmul(
                            kTp[:], lhsT=wck_sb[:, kt * HD:(kt + 1) * HD],
                            rhs=xt, **st)
                        nc.tensor.matmul(
                            vTp[:], lhsT=wcv_sb[:, kt * HD:(kt + 1) * HD],
                            rhs=xt, **st)
                    return qd, kTp, vTp

                def p1_rope(ch, qd, kTp, vTp):
                    cs = slice(ch * CH, (ch + 1) * CH)
                    # rope + bias: dst = (ps+b)*cos2 + J @ ((ps+b)*sin2)
                    # (J = [[0,-I64],[I64,0]] does the half-swap on the PE;
                    #  cross-partition DVE ops are illegal on HW)
                    for ti, (ps, dst) in enumerate(
                            [(qd[0], qT_sb[:, 0 * T + ch * CH:0 * T + (ch + 1) * CH]),
                             (qd[1], qT_sb[:, 1 * T + ch * CH:1 * T + (ch + 1) * CH]),
                             (kTp, kT_sb[:, cs])]):
                        U = rsp.tile([128, CH], F32, tag="U", name=f"U{ch}_{ti}")
                        Wt = rsp.tile([128, CH], MMDT, tag="W", name=f"Wt{ch}_{ti}")
                        b = bias3[:, ti:ti + 1]
                        nc.vector.scalar_tensor_tensor(
                            U[:], ps[:], b, cos2_sb[:, cs], op0=OP.add, op1=OP.mult)
                        nc.vector.scalar_tensor_tensor(
                            Wt[:], ps[:], b, sin2_sb[:, cs], op0=OP.add, op1=OP.mult)
                        rp = psA.tile([128, CH], F32, tag="qkT", name=f"rp{ch}_{ti}")
                        nc.tensor.matmul(rp[:], lhsT=jt_sb[:], rhs=Wt[:],
                                         start=True, stop=True)
                        nc.vector.tensor_add(dst, rp[:], U[:])
                    nc.any.tensor_copy(vT_sb[:, cs], vTp[:])

                prev = None
                for ch in range(NCH):
                    cur = p1_mms(ch)
                    if prev is not None:
                        p1_rope(ch - 1, *prev)
                    prev = cur
                p1_rope(NCH - 1, *prev)

                # v[t, d] from vT[d, t] via one xbar transpose (bf16)
                nc.sync.dma_start_transpose(
                    v_sb[:].rearrange("p (k f) -> p k f", k=KT), vT_sb[:])

                # block means of roped kT: [128, T] -> [128, NB], 1/BS scale
                with nc.allow_low_precision(reason="bf16 block-mean output"):
                    nc.vector.reduce_sum(
                        kbT[:, :, None],
                        kT_sb[:].rearrange("p (b i) -> p b i", b=NB),
                        axis=mybir.AxisListType.X)
                nc.vector.tensor_scalar_mul(kbT[:], kbT[:], 1.0 / BS)
            # bulk block scores + top-16 boost for every (h, qt) --
            # keeps the 5-op DVE chain off phase 2's critical path
            if True:
                with tc.tile_pool(name="psB", bufs=4, space="PSUM") as psB, \
                     tc.tile_pool(name="pTk", bufs=6) as pTk:
                    for h in range(HPC):
                        for qt in range(QT):
                            qTh = qT_sb[:, h * T + qt * 128:h * T + (qt + 1) * 128]
                            bsc = psB.tile([128, NB], F32, tag="bsc",
                                           name=f"bsc{qt}_{h}")
                            nc.tensor.matmul(bsc[:], lhsT=qTh, rhs=kbT[:],
                                             start=True, stop=True)
                            z = pTk.tile([128, NB], F32, tag="z", name=f"z{qt}_{h}")
                            m8 = pTk.tile([128, 8], F32, tag="m8",
                                          name=f"m8_{qt}_{h}")
                            nc.scalar.copy(z[:], bsc[:])
                            for _ in range(NSEL // 8):
                                nc.vector.max(out=m8[:], in_=z[:])
                                nc.vector.match_replace(
                                    out=z[:], in_to_replace=m8[:], in_values=z[:],
                                    imm_value=NEG)
                            bo_sl = boost_all[:, (h * QT + qt) * NB:
                                              (h * QT + qt + 1) * NB]
                            nc.vector.scalar_tensor_tensor(
                                bo_sl, z[:], NEG, bsc[:],
                                op0=OP.is_le, op1=OP.mult)

            # ---------- phase 2: attention + projection ----------
            # Stage A(qt): S matmuls -> fused boost-add copy to SBUF (frees
            # PSUM) -> masks -> exp -> normalize to bf16.
            # Stage B(qt): xbar-transpose P -> AV -> projection -> DMA out.
            # Emitted software-pipelined: A(qt) then B(qt-1), so PE always has
            # adjacent work and HAM stays warm.
            with tc.tile_pool(name="psS", bufs=2, space="PSUM") as psS, \
                 tc.tile_pool(name="psAcc", bufs=2, space="PSUM") as psAcc, \
                 tc.tile_pool(name="psPT", bufs=2, space="PSUM") as psPT, \
                 tc.tile_pool(name="pSb", bufs=6) as pSb, \
                 tc.tile_pool(name="pPn", bufs=3) as pPn, \
                 tc.tile_pool(name="pPT", bufs=2) as pPT, \
                 tc.tile_pool(name="pA", bufs=4) as pA, \
                 tc.tile_pool(name="pOut", bufs=2) as pOut, \
                 tc.tile_pool(name="pSm", bufs=8) as pSm:
                pn_tiles = {}

                def stage_a(qt):
                    nk = min(qt, 4) + 1
                    kt0 = qt + 1 - nk
                    Pn = pPn.tile([128, 2 * 640], MMDT, tag="Pn", name=f"Pn{qt}")
                    pn_tiles[qt] = Pn
                    for h in range(HPC):
                        qTh = qT_sb[:, h * T + qt * 128:h * T + (qt + 1) * 128]
                        S = psS.tile([128, 640], F32, tag="S", name=f"S{qt}_{h}")
                        o = 0
                        while o < nk * 128:          # <=512-wide, bank-aligned
                            w = min(512, nk * 128 - o)
                            nc.tensor.matmul(
                                S[:, o:o + w], lhsT=qTh,
                                rhs=kT_sb[:, kt0 * 128 + o:kt0 * 128 + o + w],
                                start=True, stop=True)
                            o += w
                        # psum->sbuf copy (frees PSUM regardless of top-k
                        # progress), then boost broadcast-add on idle GpSimd
                        Ssb = pSb.tile([128, 640], F32, tag="Ssb", name=f"Ssb{qt}_{h}")
                        nc.vector.tensor_copy(Ssb[:, :nk * 128], S[:, :nk * 128])
                        bo_sl = boost_all[:, (h * QT + qt) * NB + 2 * kt0:
                                          (h * QT + qt) * NB + 2 * (qt + 1)]
                        bv = bo_sl[:, :, None].to_broadcast([128, 2 * nk, BS])
                        Sv = Ssb[:, :nk * 128].rearrange("p (b i) -> p b i", i=BS)
                        nc.gpsimd.tensor_tensor(out=Sv, in0=Sv, in1=bv, op=OP.add)
                        # sliding-window triangle masks (in SBUF)
                        if qt >= 4:
                            nc.vector.tensor_add(Ssb[:, 0:128], Ssb[:, 0:128],
                                                 tri_hi[:])
                        nc.vector.tensor_add(
                            Ssb[:, (nk - 1) * 128:nk * 128],
                            Ssb[:, (nk - 1) * 128:nk * 128], tri_lo[:])
                        # exp (scores bounded; no row-max) + row sums
                        Praw = pSb.tile([128, 640], F32, tag="P", name=f"P{qt}_{h}")
                        r = pSm.tile([128, 1], F32, tag="r", name=f"r{qt}_{h}")
                        rinv = pSm.tile([128, 1], F32, tag="rinv",
                                        name=f"rinv{qt}_{h}")
                        nc.scalar.activation(
                            Praw[:, :nk * 128], Ssb[:, :nk * 128], AF.Exp,
                            scale=float(SCALE), accum_out=r[:])
                        nc.vector.reciprocal(rinv[:], r[:])
                        # normalize + bf16 cast on ACT
                        nc.scalar.activation(
                            Pn[:, h * nk * 128:(h + 1) * nk * 128],
                            Praw[:, :nk * 128], AF.Copy, scale=rinv[:])

                def stage_b(qt):
                    nk = min(qt, 4) + 1
                    kt0 = qt + 1 - nk
                    Pn = pn_tiles.pop(qt)
                    PnT = pPT.tile([128, 2 * 640], MMDT, tag="PnT", name=f"PnT{qt}")
                    for hj in range(2 * nk):
                        ptp = psPT.tile([128, 128], MMDT, tag="ptp",
                                        name=f"ptp{qt}_{hj}")
                        nc.tensor.transpose(
                            ptp[:], Pn[:, hj * 128:(hj + 1) * 128], eyeb[:])
                        nc.any.tensor_copy(
                            PnT[:, hj * 128:(hj + 1) * 128], ptp[:])
                    a_sb = []
                    for h in range(HPC):
                        attnT = psAcc.tile([128, 128], F32, tag="acc",
                                           name=f"attnT{qt}_{h}")
                        for j in range(nk):
                            nc.tensor.matmul(
                                attnT[:],
                                lhsT=v_sb[:, (kt0 + j) * 128:(kt0 + j + 1) * 128],
                                rhs=PnT[:, (h * nk + j) * 128:(h * nk + j + 1) * 128],
                                start=(j == 0), stop=(j == nk - 1))
                        at = pA.tile([128, 128], MMDT, tag="at", name=f"at{qt}_{h}")
                        nc.any.tensor_copy(at[:], attnT[:])
                        a_sb.append(at[:])
                    ot = pOut.tile([128, D], mybir.dt.bfloat16, tag="ot",
                                   name=f"ot{qt}")
                    for e in range(4):
                        pr = psAcc.tile([128, 512], F32, tag="acc",
                                        name=f"pr{qt}_{e}")
                        for h in range(HPC):
                            nc.tensor.matmul(
                                pr[:], lhsT=a_sb[h],
                                rhs=wo_sb[:, h * D + e * 512:h * D + (e + 1) * 512],
                                start=(h == 0), stop=(h == HPC - 1))
                        nc.any.tensor_copy(ot[:, e * 512:(e + 1) * 512], pr[:])
                    nc.sync.dma_start(out_d[qt * 128:(qt + 1) * 128, :], ot[:])

                for qt in range(QT + 1):
                    if qt < QT:
                        stage_a(qt)
                    if qt >= 1:
                        stage_b(qt - 1)
    nc.compile()
    return nc


# ---------------------------------------------------------------- host side

def _np_mm(a):
    """Convert a host array to the matmul wire dtype."""
    import concourse.mybir as _mb
    return np.ascontiguousarray(a).astype(_mb.dt.np(MMDT))


def _host_prep(x, Wc, bc, Wk, bk, Wv, bv, Wq, bq, Wo, bo, loop_idx):
    f = np.float32
    x = np.asarray(x, f).reshape(T, D)
    Wc, Wk, Wv, Wq, Wo = (np.asarray(a, f) for a in (Wc, Wk, Wv, Wq, Wo))
    bc, bk, bv, bq, bo = (np.asarray(a, f) for a in (bc, bk, bv, bq, bo))
    li = int(np.asarray(loop_idx))

    # pack [D, n] weight (D = KT*128 contraction) into SBUF layout [128, KT*n]
    def pack_w(w):
        n = w.shape[1]
        return np.ascontiguousarray(
            w.reshape(KT, 128, n).transpose(1, 0, 2).reshape(128, KT * n))

    xT = x.T  # [D, T]
    # xt_sb layout: [128, NCH, KT, CH] (chunk-major)
    xp = np.ascontiguousarray(
        xT.reshape(KT, 128, NCH, CH).transpose(1, 2, 0, 3)
        .reshape(128, NCH * KT * CH))
    Wck = Wc @ Wk
    bck = bc @ Wk + bk
    Wcv = Wc @ Wv
    bcv = bc @ Wv + bv

    pos = (np.arange(T) + li * T).astype(np.float64)
    inv = 1.0 / (10000.0 ** (np.arange(0, HD, 2).astype(np.float64) / HD))
    ang = pos[:, None] * inv[None, :]                       # [T, 64]
    cos = np.cos(ang).astype(f)
    sin = np.sin(ang).astype(f)
    cos2 = np.ascontiguousarray(np.concatenate([cos, cos], axis=1).T)  # [128, T]
    sin2 = np.ascontiguousarray(np.concatenate([sin, sin], axis=1).T)

    perm = np.concatenate([np.arange(0, HD, 2), np.arange(1, HD, 2)])

    a = np.arange(128)
    tri_lo = np.where(a[None, :] <= a[:, None], 0.0, MASKV).astype(f)  # causal
    tri_hi = np.where(a[None, :] >= a[:, None], 0.0, MASKV).astype(f)
    eye = np.eye(128, dtype=f)
    J = np.zeros((128, 128), f)
    J[np.arange(64), np.arange(64) + 64] = -1.0
    J[np.arange(64) + 64, np.arange(64)] = 1.0
    jt = np.ascontiguousarray(J.T)

    in_maps = []
    bo_eff = bo.copy()
    for c in range(NCORE):
        h0 = HPC * c
        g = h0 // (NH // NKV)
        Wq_c = Wq[:, h0 * HD:(h0 + HPC) * HD].reshape(D, HPC, HD)[:, :, perm]
        Wq_c = np.ascontiguousarray(Wq_c.reshape(D, HPC * HD))
        bq_c = bq[h0 * HD:(h0 + HPC) * HD].reshape(HPC, HD)[:, perm]
        Wck_c = np.ascontiguousarray(Wck[:, g * HD:(g + 1) * HD][:, perm])
        bck_c = bck[g * HD:(g + 1) * HD][perm]
        Wcv_c = np.ascontiguousarray(Wcv[:, g * HD:(g + 1) * HD])
        bcv_c = bcv[g * HD:(g + 1) * HD]
        Wo_c = Wo[h0 * HD:(h0 + HPC) * HD, :]
        # v-bias folded through softmax (rows sum to 1): + bcv @ Wo_head, per head
        for hh in range(HPC):
            bo_eff = bo_eff + bcv_c @ Wo_c[hh * HD:(hh + 1) * HD]
        # wo_sb layout [128, HPC*D]: wo[p, h*D+j] = Wo_c[h*HD+p, j]
        Wo_p = np.ascontiguousarray(
            Wo_c.reshape(HPC, HD, D).transpose(1, 0, 2).reshape(128, HPC * D))
        bias3 = np.stack([bq_c[0], bq_c[1], bck_c], axis=1).astype(f)  # [128, 3]
        in_maps.append({
            "xT": _np_mm(xp), "wq": _np_mm(pack_w(Wq_c)),
            "wck": _np_mm(pack_w(Wck_c)), "wcv": _np_mm(pack_w(Wcv_c)),
            "wo": _np_mm(Wo_p),
            "cos2": cos2, "sin2": sin2, "tri_lo": tri_lo, "tri_hi": tri_hi,
            "eye": eye, "jt": _np_mm(jt), "bias3": bias3,
        })
    return in_maps, bo_eff


def _maybe_install_ntff_hook():
    """This axon image lacks antenv.axon_hooks; synthesize it so
    run_bass_kernel_spmd(trace=True) can capture NTFFs. Best-effort."""
    try:
        import sys
        import types
        import antenv
        if getattr(antenv, "axon_hooks", None) is not None:
            return
        from trn_agent_boot.trn_boot import _ntff_profile_via_ctypes
        hook = _ntff_profile_via_ctypes("/opt/axon/libaxon_pjrt.so")
        mod = types.ModuleType("antenv.axon_hooks")
        mod._hook = hook
        mod.get_axon_ntff_profile_hook = lambda: mod._hook
        mod.set_axon_ntff_profile_hook = lambda h: setattr(mod, "_hook", h)
        sys.modules["antenv.axon_hooks"] = mod
        antenv.axon_hooks = mod
    except Exception as e:  # profiling is optional
        print(f"ntff hook install failed: {e}")


def kernel(**inputs) -> np.ndarray:
    in_maps, bo_eff = _host_prep(**inputs)
    if "nc" not in _cache:
        _cache["nc"] = build_nc()
    trace = bool(int(os.environ.get("KERNEL_TRACE", "0")))
    if trace:
        _maybe_install_ntff_hook()
    res = run_bass_kernel_spmd(
        _cache["nc"], in_maps, core_ids=list(range(NCORE)),
        trace=trace)
    if trace:
        _cache["last_results"] = res
    out = np.zeros((T, D), np.float64)
    for r in res.results:
        out += r["out"].astype(np.float64)
    out = (out + bo_eff.astype(np.float64)).astype(np.float32)
    return out.reshape(B, T, D)



# revision 9
# speedup vs baseline: 1.0116x; 1.0013x over previous
"""Trainium2 Bass kernel for nn_DSA2Attention (MLA-latent sparse sliding-window attention).

Strategy (tensor-parallel over heads, 8 cores, 2 heads/core):
  host:  fold Wc into Wk/Wv (k = x @ (Wc@Wk) etc), permute q/k head-dims so rope
         pairs become [x1(0:64); x2(64:128)], precompute rope cos/sin tables in
         [d', t] layout, sliding-window triangle masks, identity.
  device (per core, SPMD — identical program, different weight slices):
    phase 1: qT[d,t], kT[d,t] (feature-major) and v[t,d] via PE matmuls from
             xT chunks; rope on DVE; block-mean kbT via segmented reduce.
    phase 2: per query-tile of 128: S = qT.T@kT over <=5 key tiles (sliding
             window) + bsc = qT.T@kbT in one PSUM tile; top-16-of-32 block
             boost via max8/match_replace; boost broadcast-add + triangle
             masks; exp (no row-max needed: scores bounded) with accumulated
             row sums; P.T via PE "transpose" against diag(1/r) (folds the
             softmax normalization in for free); AV accumulate attnT[d,q];
             out-projection psum -> DMA straight to DRAM.
  host:  sum the 8 partial projections (row-parallel Wo) + bias.

Numerics: matmul operands in bf16 (fp32 PSUM accumulation), softmax chain in
fp32, output partials bf16 summed in fp64 on host. Measured relative error vs
the fp32 reference: ~3.7e-3 (absmax-relative). Masks use -1e30/SCALE additive.
Set MM_DT=f32 for an fp32-exact (~2.3x slower) variant.
"""
import os
import numpy as np

import concourse.bacc as bacc
import concourse.bass as bass
import concourse.mybir as mybir
import concourse.tile as tile
from concourse.bass_utils import run_bass_kernel_spmd

B, T, D = 1, 2048, 2048
NH, NKV, HD = 16, 4, 128
KVC = 512
WIN = 512
BS = 64
NSEL = 16
SCALE = HD ** -0.5
NB = T // BS          # 32
NCORE = 8
HPC = NH // NCORE     # heads per core = 2

KT = T // 128         # 16 k-tiles
NCH = 4               # phase-1 t-chunks
CH = T // NCH         # 512
QT = T // 128         # 16 query tiles
NEG = -1e30
MASKV = -1e30 / SCALE

F32 = mybir.dt.float32
R32 = mybir.dt.float32r
AF = mybir.ActivationFunctionType
OP = mybir.AluOpType

# dtype for every tensor that feeds a matmul. bf16: 1 cyc/row + fast weight
# load (fp32: 2 cyc/row, exact; fp32r crashes the exec unit on this toolchain).
MM_DT = os.environ.get("MM_DT", "bf16")
MMDT = {"bf16": mybir.dt.bfloat16, "f32": F32, "f32r": R32}[MM_DT]
NPMM = None  # numpy dtype for host-side arrays feeding matmuls; set below


def _mr(ap):
    return ap

_cache = {}


def build_nc(trace_label=""):
    nc = bacc.Bacc("TRN2", target_bir_lowering=False, debug=False, num_devices=NCORE)

    # Host-prepacked to the exact SBUF layouts: every DMA row is contiguous
    # DRAM (few large descriptors instead of thousands of 512B ones).
    xT_d = nc.dram_tensor("xT", [128, NCH * KT * CH], MMDT,
                          kind="ExternalInput").ap()
    wq_d = nc.dram_tensor("wq", [128, KT * HPC * HD], MMDT,
                          kind="ExternalInput").ap()
    wck_d = nc.dram_tensor("wck", [128, KT * HD], MMDT,
                           kind="ExternalInput").ap()
    wcv_d = nc.dram_tensor("wcv", [128, KT * HD], MMDT,
                           kind="ExternalInput").ap()
    wo_d = nc.dram_tensor("wo", [128, HPC * D], MMDT, kind="ExternalInput").ap()
    cos2_d = nc.dram_tensor("cos2", [HD, T], F32, kind="ExternalInput").ap()
    sin2_d = nc.dram_tensor("sin2", [HD, T], F32, kind="ExternalInput").ap()
    tri_lo_d = nc.dram_tensor("tri_lo", [128, 128], F32, kind="ExternalInput").ap()
    tri_hi_d = nc.dram_tensor("tri_hi", [128, 128], F32, kind="ExternalInput").ap()
    eye_d = nc.dram_tensor("eye", [128, 128], F32, kind="ExternalInput").ap()
    jt_d = nc.dram_tensor("jt", [128, 128], MMDT, kind="ExternalInput").ap()
    bias3_d = nc.dram_tensor("bias3", [HD, 3], F32, kind="ExternalInput").ap()
    out_d = nc.dram_tensor("out", [T, D], mybir.dt.bfloat16,
                           kind="ExternalOutput").ap()

    with tile.TileContext(nc) as tc:
        with tc.tile_pool(name="persist", bufs=1) as pp:
            wq_sb = pp.tile([128, KT * HPC * HD], MMDT, tag="wq")
            wck_sb = pp.tile([128, KT * HD], MMDT, tag="wck")
            wcv_sb = pp.tile([128, KT * HD], MMDT, tag="wcv")
            wo_sb = pp.tile([128, HPC * D], MMDT, tag="wo")
            cos2_sb = pp.tile([128, T], F32, tag="cos2")
            sin2_sb = pp.tile([128, T], F32, tag="sin2")
            tri_lo = pp.tile([128, 128], F32, tag="tri_lo")
            tri_hi = pp.tile([128, 128], F32, tag="tri_hi")
            eye_sb = pp.tile([128, 128], F32, tag="eye")
            jt_sb = pp.tile([128, 128], MMDT, tag="jt")
            eyeb = pp.tile([128, 128], MMDT, tag="eyeb")
            bias3 = pp.tile([128, 3], F32, tag="bias3")
            qT_sb = pp.tile([128, HPC * T], MMDT, tag="qT")
            kT_sb = pp.tile([128, T], MMDT, tag="kT")
            v_sb = pp.tile([128, KT * HD], MMDT, tag="v")
            kbT = pp.tile([128, NB], MMDT, tag="kbT")

            # PE warmup: trip the HAM clock-gate to 8/8 while the initial
            # DMAs are in flight so real matmuls start at 2.4 GHz.
            warm_sb = pp.tile([128, 512], MMDT, tag="warm")
            with tc.tile_pool(name="psW", bufs=2, space="PSUM") as psW:
                NWARM = 20
                for i in range(NWARM):
                    wps = psW.tile([128, 512], F32, tag="w", name=f"warm{i}")
                    nc.tensor.matmul(wps[:], lhsT=warm_sb[:, :128],
                                     rhs=warm_sb[:], start=True, stop=True)
                    if i == NWARM - 1:
                        nc.vector.tensor_copy(warm_sb[:, :512], wps[:])

            # weight DMAs split into <=256KB pieces so they spread across the
            # 16 DMA queues (a single dma_start runs on ONE queue at ~100GB/s)
            def dma_split(dst, src, ncols, npiece):
                w = ncols // npiece
                for i in range(npiece):
                    nc.sync.dma_start(dst[:, i * w:(i + 1) * w],
                                      src[:, i * w:(i + 1) * w])
            dma_split(wq_sb, wq_d, KT * HPC * HD, 8)
            dma_split(wck_sb, wck_d, KT * HD, 4)
            dma_split(wcv_sb, wcv_d, KT * HD, 4)

            # ---------- phase 1: qT, kT, vT ----------
            # xT stays resident in SBUF (bf16: 64KB/partition), ch-major.
            xt_sb = pp.tile([128, NCH * KT * CH], MMDT, tag="xt")
            vT_sb = pp.tile([128, T], MMDT, tag="vT")
            boost_all = pp.tile([128, QT * HPC * NB], F32, tag="boost_all")
            for i in range(8):
                w = KT * CH // 8
                nc.sync.dma_start(xt_sb[:, i * w:(i + 1) * w],
                                  xT_d[:, i * w:(i + 1) * w])
            nc.sync.dma_start(bias3[:], bias3_d)
            dma_split(cos2_sb, cos2_d, T, 4)
            dma_split(sin2_sb, sin2_d, T, 4)
            nc.sync.dma_start(jt_sb[:], jt_d)
            nc.sync.dma_start(tri_lo[:], tri_lo_d)
            nc.sync.dma_start(tri_hi[:], tri_hi_d)
            nc.sync.dma_start(eye_sb[:], eye_d)
            nc.vector.tensor_copy(eyeb[:], eye_sb[:])
            for ch in range(1, NCH):
                for i in range(4):
                    w = KT * CH // 4
                    o = ch * KT * CH + i * w
                    nc.sync.dma_start(xt_sb[:, o:o + w], xT_d[:, o:o + w])
            dma_split(wo_sb, wo_d, HPC * D, 4)
            with tc.tile_pool(name="rs", bufs=3) as rsp, \
                 tc.tile_pool(name="psA", bufs=8, space="PSUM") as psA:
                def p1_mms(ch):
                    qd = [psA.tile([128, CH], F32, tag="qkT", name=f"qd{ch}_{_h}")
                          for _h in range(HPC)]
                    kTp = psA.tile([128, CH], F32, tag="qkT", name=f"kTp{ch}")
                    vTp = psA.tile([128, CH], F32, tag="qkT", name=f"vTp{ch}")
                    for kt in range(KT):
                        xt = xt_sb[:, (ch * KT + kt) * CH:(ch * KT + kt + 1) * CH]
                        st = dict(start=(kt == 0), stop=(kt == KT - 1))
                        for h in range(HPC):
                            nc.tensor.matmul(
                                qd[h][:],
                                lhsT=wq_sb[:, kt * HPC * HD + h * HD:
                                           kt * HPC * HD + (h + 1) * HD],
                                rhs=xt, **st)
                        nc.tensor.matmul(
                            kTp[:], lhsT=wck_sb[:, kt * HD:(kt + 1) * HD],
                            rhs=xt, **st)
                        nc.tensor.matmul(
                            vTp[:], lhsT=wcv_sb[:, kt * HD:(kt + 1) * HD],
                            rhs=xt, **st)
                    return qd, kTp, vTp

                def p1_rope(ch, qd, kTp, vTp):
                    cs = slice(ch * CH, (ch + 1) * CH)
                    # rope + bias: dst = (ps+b)*cos2 + J @ ((ps+b)*sin2)
                    # (J = [[0,-I64],[I64,0]] does the half-swap on the PE;
                    #  cross-partition DVE ops are illegal on HW)
                    for ti, (ps, dst) in enumerate(
                            [(qd[0], qT_sb[:, 0 * T + ch * CH:0 * T + (ch + 1) * CH]),
                             (qd[1], qT_sb[:, 1 * T + ch * CH:1 * T + (ch + 1) * CH]),
                             (kTp, kT_sb[:, cs])]):
                        U = rsp.tile([128, CH], F32, tag="U", name=f"U{ch}_{ti}")
                        Wt = rsp.tile([128, CH], MMDT, tag="W", name=f"Wt{ch}_{ti}")
                        b = bias3[:, ti:ti + 1]
                        nc.vector.scalar_tensor_tensor(
                            U[:], ps[:], b, cos2_sb[:, cs], op0=OP.add, op1=OP.mult)
                        nc.vector.scalar_tensor_tensor(
                            Wt[:], ps[:], b, sin2_sb[:, cs], op0=OP.add, op1=OP.mult)
                        rp = psA.tile([128, CH], F32, tag="qkT", name=f"rp{ch}_{ti}")
                        nc.tensor.matmul(rp[:], lhsT=jt_sb[:], rhs=Wt[:],
                                         start=True, stop=True)
                        nc.vector.tensor_add(dst, rp[:], U[:])
                    nc.any.tensor_copy(vT_sb[:, cs], vTp[:])

                prev = None
                for ch in range(NCH):
                    cur = p1_mms(ch)
                    if prev is not None:
                        p1_rope(ch - 1, *prev)
                    prev = cur
                p1_rope(NCH - 1, *prev)

                # v[t, d] from vT[d, t] via one xbar transpose (bf16)
                nc.sync.dma_start_transpose(
                    v_sb[:].rearrange("p (k f) -> p k f", k=KT), vT_sb[:])

                # block means of roped kT: [128, T] -> [128, NB], 1/BS scale
                with nc.allow_low_precision(reason="bf16 block-mean output"):
                    nc.vector.reduce_sum(
                        kbT[:, :, None],
                        kT_sb[:].rearrange("p (b i) -> p b i", b=NB),
                        axis=mybir.AxisListType.X)
                nc.vector.tensor_scalar_mul(kbT[:], kbT[:], 1.0 / BS)
            # bulk block scores + top-16 boost for every (h, qt) --
            # keeps the 5-op DVE chain off phase 2's critical path
            if True:
                with tc.tile_pool(name="psB", bufs=4, space="PSUM") as psB, \
                     tc.tile_pool(name="pTk", bufs=6) as pTk:
                    for h in range(HPC):
                        for qt in range(QT):
                            qTh = qT_sb[:, h * T + qt * 128:h * T + (qt + 1) * 128]
                            bsc = psB.tile([128, NB], F32, tag="bsc",
                                           name=f"bsc{qt}_{h}")
                            nc.tensor.matmul(bsc[:], lhsT=qTh, rhs=kbT[:],
                                             start=True, stop=True)
                            z = pTk.tile([128, NB], F32, tag="z", name=f"z{qt}_{h}")
                            m8 = pTk.tile([128, 8], F32, tag="m8",
                                          name=f"m8_{qt}_{h}")
                            nc.scalar.copy(z[:], bsc[:])
                            for _ in range(NSEL // 8):
                                nc.vector.max(out=m8[:], in_=z[:])
                                nc.vector.match_replace(
                                    out=z[:], in_to_replace=m8[:], in_values=z[:],
                                    imm_value=NEG)
                            bo_sl = boost_all[:, (h * QT + qt) * NB:
                                              (h * QT + qt + 1) * NB]
                            nc.vector.scalar_tensor_tensor(
                                bo_sl, z[:], NEG, bsc[:],
                                op0=OP.is_le, op1=OP.mult)

            # ---------- phase 2: attention + projection ----------
            # Stage A(qt): S matmuls -> fused boost-add copy to SBUF (frees
            # PSUM) -> masks -> exp -> normalize to bf16.
            # Stage B(qt): xbar-transpose P -> AV -> projection -> DMA out.
            # Emitted software-pipelined: A(qt) then B(qt-1), so PE always has
            # adjacent work and HAM stays warm.
            with tc.tile_pool(name="psS", bufs=2, space="PSUM") as psS, \
                 tc.tile_pool(name="psAcc", bufs=2, space="PSUM") as psAcc, \
                 tc.tile_pool(name="psPT", bufs=2, space="PSUM") as psPT, \
                 tc.tile_pool(name="pSb", bufs=6) as pSb, \
                 tc.tile_pool(name="pPn", bufs=3) as pPn, \
                 tc.tile_pool(name="pPT", bufs=2) as pPT, \
                 tc.tile_pool(name="pA", bufs=4) as pA, \
                 tc.tile_pool(name="pOut", bufs=2) as pOut, \
                 tc.tile_pool(name="pSm", bufs=8) as pSm:
                pn_tiles = {}

                def stage_a(qt):
                    nk = min(qt, 4) + 1
                    kt0 = qt + 1 - nk
                    Pn = pPn.tile([128, 2 * 640], MMDT, tag="Pn", name=f"Pn{qt}")
                    pn_tiles[qt] = Pn
                    for h in range(HPC):
                        qTh = qT_sb[:, h * T + qt * 128:h * T + (qt + 1) * 128]
                        S = psS.tile([128, 640], F32, tag="S", name=f"S{qt}_{h}")
                        o = 0
                        while o < nk * 128:          # <=512-wide, bank-aligned
                            w = min(512, nk * 128 - o)
                            nc.tensor.matmul(
                                S[:, o:o + w], lhsT=qTh,
                                rhs=kT_sb[:, kt0 * 128 + o:kt0 * 128 + o + w],
                                start=True, stop=True)
                            o += w
                        # psum->sbuf copy (frees PSUM regardless of top-k
                        # progress), then boost broadcast-add on idle GpSimd
                        Ssb = pSb.tile([128, 640], F32, tag="Ssb", name=f"Ssb{qt}_{h}")
                        nc.vector.tensor_copy(Ssb[:, :nk * 128], S[:, :nk * 128])
                        bo_sl = boost_all[:, (h * QT + qt) * NB + 2 * kt0:
                                          (h * QT + qt) * NB + 2 * (qt + 1)]
                        bv = bo_sl[:, :, None].to_broadcast([128, 2 * nk, BS])
                        Sv = Ssb[:, :nk * 128].rearrange("p (b i) -> p b i", i=BS)
                        nc.gpsimd.tensor_tensor(out=Sv, in0=Sv, in1=bv, op=OP.add)
                        # sliding-window triangle masks (in SBUF)
                        if qt >= 4:
                            nc.vector.tensor_add(Ssb[:, 0:128], Ssb[:, 0:128],
                                                 tri_hi[:])
                        nc.vector.tensor_add(
                            Ssb[:, (nk - 1) * 128:nk * 128],
                            Ssb[:, (nk - 1) * 128:nk * 128], tri_lo[:])
                        # exp (scores bounded; no row-max) + row sums
                        Praw = pSb.tile([128, 640], F32, tag="P", name=f"P{qt}_{h}")
                        r = pSm.tile([128, 1], F32, tag="r", name=f"r{qt}_{h}")
                        rinv = pSm.tile([128, 1], F32, tag="rinv",
                                        name=f"rinv{qt}_{h}")
                        nc.scalar.activation(
                            Praw[:, :nk * 128], Ssb[:, :nk * 128], AF.Exp,
                            scale=float(SCALE), accum_out=r[:])
                        nc.vector.reciprocal(rinv[:], r[:])
                        # normalize + bf16 cast on ACT
                        nc.scalar.activation(
                            Pn[:, h * nk * 128:(h + 1) * nk * 128],
                            Praw[:, :nk * 128], AF.Copy, scale=rinv[:])

                def stage_b(qt):
                    nk = min(qt, 4) + 1
                    kt0 = qt + 1 - nk
                    Pn = pn_tiles.pop(qt)
                    PnT = pPT.tile([128, 2 * 640], MMDT, tag="PnT", name=f"PnT{qt}")
                    for hj in range(2 * nk):
                        ptp = psPT.tile([128, 128], MMDT, tag="ptp",
                                        name=f"ptp{qt}_{hj}")
                        nc.tensor.transpose(
                            ptp[:], Pn[:, hj * 128:(hj + 1) * 128], eyeb[:])
                        nc.any.tensor_copy(
                            PnT[:, hj * 128:(hj + 1) * 128], ptp[:])
                    a_sb = []
                    for h in range(HPC):
                        attnT = psAcc.tile([128, 128], F32, tag="acc",
                                           name=f"attnT{qt}_{h}")
                        for j in range(nk):
                            nc.tensor.matmul(
                                attnT[:],
                                lhsT=v_sb[:, (kt0 + j) * 128:(kt0 + j + 1) * 128],
                                rhs=PnT[:, (h * nk + j) * 128:(h * nk + j + 1) * 128],
                                start=(j == 0), stop=(j == nk - 1))
                        at = pA.tile([128, 128], MMDT, tag="at", name=f"at{qt}_{h}")
                        nc.any.tensor_copy(at[:], attnT[:])
                        a_sb.append(at[:])
                    ot = pOut.tile([128, D], mybir.dt.bfloat16, tag="ot",
                                   name=f"ot{qt}")
                    for e in range(4):
                        pr = psAcc.tile([128, 512], F32, tag="acc",
                                        name=f"pr{qt}_{e}")
                        for h in range(HPC):
                            nc.tensor.matmul(
                                pr[:], lhsT=a_sb[h],
                                rhs=wo_sb[:, h * D + e * 512:h * D + (e + 1) * 512],
                                start=(h == 0), stop=(h == HPC - 1))
                        nc.any.tensor_copy(ot[:, e * 512:(e + 1) * 512], pr[:])
                    nc.sync.dma_start(out_d[qt * 128:(qt + 1) * 128, :], ot[:])

                for qt in range(QT + 1):
                    if qt < QT:
                        stage_a(qt)
                    if qt >= 1:
                        stage_b(qt - 1)
    nc.compile()
    return nc


# ---------------------------------------------------------------- host side

def _np_mm(a):
    """Convert a host array to the matmul wire dtype."""
    import concourse.mybir as _mb
    return np.ascontiguousarray(a).astype(_mb.dt.np(MMDT))


def _host_prep(x, Wc, bc, Wk, bk, Wv, bv, Wq, bq, Wo, bo, loop_idx):
    f = np.float32
    x = np.asarray(x, f).reshape(T, D)
    Wc, Wk, Wv, Wq, Wo = (np.asarray(a, f) for a in (Wc, Wk, Wv, Wq, Wo))
    bc, bk, bv, bq, bo = (np.asarray(a, f) for a in (bc, bk, bv, bq, bo))
    li = int(np.asarray(loop_idx))

    # pack [D, n] weight (D = KT*128 contraction) into SBUF layout [128, KT*n]
    def pack_w(w):
        n = w.shape[1]
        return np.ascontiguousarray(
            w.reshape(KT, 128, n).transpose(1, 0, 2).reshape(128, KT * n))

    xT = x.T  # [D, T]
    # xt_sb layout: [128, NCH, KT, CH] (chunk-major)
    xp = np.ascontiguousarray(
        xT.reshape(KT, 128, NCH, CH).transpose(1, 2, 0, 3)
        .reshape(128, NCH * KT * CH))
    Wck = Wc @ Wk
    bck = bc @ Wk + bk
    Wcv = Wc @ Wv
    bcv = bc @ Wv + bv

    pos = (np.arange(T) + li * T).astype(np.float64)
    inv = 1.0 / (10000.0 ** (np.arange(0, HD, 2).astype(np.float64) / HD))
    ang = pos[:, None] * inv[None, :]                       # [T, 64]
    cos = np.cos(ang).astype(f)
    sin = np.sin(ang).astype(f)
    cos2 = np.ascontiguousarray(np.concatenate([cos, cos], axis=1).T)  # [128, T]
    sin2 = np.ascontiguousarray(np.concatenate([sin, sin], axis=1).T)

    perm = np.concatenate([np.arange(0, HD, 2), np.arange(1, HD, 2)])

    a = np.arange(128)
    tri_lo = np.where(a[None, :] <= a[:, None], 0.0, MASKV).astype(f)  # causal
    tri_hi = np.where(a[None, :] >= a[:, None], 0.0, MASKV).astype(f)
    eye = np.eye(128, dtype=f)
    J = np.zeros((128, 128), f)
    J[np.arange(64), np.arange(64) + 64] = -1.0
    J[np.arange(64) + 64, np.arange(64)] = 1.0
    jt = np.ascontiguousarray(J.T)

    in_maps = []
    bo_eff = bo.copy()
    for c in range(NCORE):
        h0 = HPC * c
        g = h0 // (NH // NKV)
        Wq_c = Wq[:, h0 * HD:(h0 + HPC) * HD].reshape(D, HPC, HD)[:, :, perm]
        Wq_c = np.ascontiguousarray(Wq_c.reshape(D, HPC * HD))
        bq_c = bq[h0 * HD:(h0 + HPC) * HD].reshape(HPC, HD)[:, perm]
        Wck_c = np.ascontiguousarray(Wck[:, g * HD:(g + 1) * HD][:, perm])
        bck_c = bck[g * HD:(g + 1) * HD][perm]
        Wcv_c = np.ascontiguousarray(Wcv[:, g * HD:(g + 1) * HD])
        bcv_c = bcv[g * HD:(g + 1) * HD]
        Wo_c = Wo[h0 * HD:(h0 + HPC) * HD, :]
        # v-bias folded through softmax (rows sum to 1): + bcv @ Wo_head, per head
        for hh in range(HPC):
            bo_eff = bo_eff + bcv_c @ Wo_c[hh * HD:(hh + 1) * HD]
        # wo_sb layout [128, HPC*D]: wo[p, h*D+j] = Wo_c[h*HD+p, j]
        Wo_p = np.ascontiguousarray(
            Wo_c.reshape(HPC, HD, D).transpose(1, 0, 2).reshape(128, HPC * D))
        bias3 = np.stack([bq_c[0], bq_c[1], bck_c], axis=1).astype(f)  # [128, 3]
        in_maps.append({
            "xT": _np_mm(xp), "wq": _np_mm(pack_w(Wq_c)),
            "wck": _np_mm(pack_w(Wck_c)), "wcv": _np_mm(pack_w(Wcv_c)),
            "wo": _np_mm(Wo_p),
            "cos2": cos2, "sin2": sin2, "tri_lo": tri_lo, "tri_hi": tri_hi,
            "eye": eye, "jt": _np_mm(jt), "bias3": bias3,
        })
    return in_maps, bo_eff


def _maybe_install_ntff_hook():
    """This axon image lacks antenv.axon_hooks; synthesize it so
    run_bass_kernel_spmd(trace=True) can capture NTFFs. Best-effort."""
    try:
        import sys
        import types
        import antenv
        if getattr(antenv, "axon_hooks", None) is not None:
            return
        from trn_agent_boot.trn_boot import _ntff_profile_via_ctypes
        hook = _ntff_profile_via_ctypes("/opt/axon/libaxon_pjrt.so")
        mod = types.ModuleType("antenv.axon_hooks")
        mod._hook = hook
        mod.get_axon_ntff_profile_hook = lambda: mod._hook
        mod.set_axon_ntff_profile_hook = lambda h: setattr(mod, "_hook", h)
        sys.modules["antenv.axon_hooks"] = mod
        antenv.axon_hooks = mod
    except Exception as e:  # profiling is optional
        print(f"ntff hook install failed: {e}")


def kernel(**inputs) -> np.ndarray:
    in_maps, bo_eff = _host_prep(**inputs)
    if "nc" not in _cache:
        _cache["nc"] = build_nc()
    trace = bool(int(os.environ.get("KERNEL_TRACE", "0")))
    if trace:
        _maybe_install_ntff_hook()
    res = run_bass_kernel_spmd(
        _cache["nc"], in_maps, core_ids=list(range(NCORE)),
        trace=trace)
    if trace:
        _cache["last_results"] = res
    out = np.zeros((T, D), np.float64)
    for r in res.results:
        out += r["out"].astype(np.float64)
    out = (out + bo_eff.astype(np.float64)).astype(np.float32)
    return out.reshape(B, T, D)



# revision 12
# speedup vs baseline: 1.0700x; 1.0577x over previous
"""Trainium2 Bass kernel for nn_DSA2Attention (MLA-latent sparse sliding-window attention).

Strategy (tensor-parallel over heads, 8 cores, 2 heads/core):
  host:  fold Wc into Wk/Wv (k = x @ (Wc@Wk) etc), permute q/k head-dims so rope
         pairs become [x1(0:64); x2(64:128)], precompute rope cos/sin tables in
         [d', t] layout, sliding-window triangle masks, identity.
  device (per core, SPMD — identical program, different weight slices):
    phase 1: qT[d,t], kT[d,t] (feature-major) and v[t,d] via PE matmuls from
             xT chunks; rope on DVE; block-mean kbT via segmented reduce.
    phase 2: per query-tile of 128: S = qT.T@kT over <=5 key tiles (sliding
             window) + bsc = qT.T@kbT in one PSUM tile; top-16-of-32 block
             boost via max8/match_replace; boost broadcast-add + triangle
             masks; exp (no row-max needed: scores bounded) with accumulated
             row sums; P.T via PE "transpose" against diag(1/r) (folds the
             softmax normalization in for free); AV accumulate attnT[d,q];
             out-projection psum -> DMA straight to DRAM.
  host:  sum the 8 partial projections (row-parallel Wo) + bias.

Numerics: matmul operands in bf16 (fp32 PSUM accumulation), softmax chain in
fp32, output partials bf16 summed in fp64 on host. Measured relative error vs
the fp32 reference: ~3.7e-3 (absmax-relative). Masks use -1e30/SCALE additive.
Set MM_DT=f32 for an fp32-exact (~2.3x slower) variant.
"""
import os
import numpy as np

import concourse.bacc as bacc
import concourse.bass as bass
import concourse.mybir as mybir
import concourse.tile as tile
from concourse.bass_utils import run_bass_kernel_spmd

B, T, D = 1, 2048, 2048
NH, NKV, HD = 16, 4, 128
KVC = 512
WIN = 512
BS = 64
NSEL = 16
SCALE = HD ** -0.5
NB = T // BS          # 32
NCORE = 8
HPC = NH // NCORE     # heads per core = 2

KT = T // 128         # 16 k-tiles
NCH = 4               # phase-1 t-chunks
CH = T // NCH         # 512
QT = T // 128         # 16 query tiles
NEG = -1e30
MASKV = -1e30 / SCALE

F32 = mybir.dt.float32
R32 = mybir.dt.float32r
AF = mybir.ActivationFunctionType
OP = mybir.AluOpType

# dtype for every tensor that feeds a matmul. bf16: 1 cyc/row + fast weight
# load (fp32: 2 cyc/row, exact; fp32r crashes the exec unit on this toolchain).
MM_DT = os.environ.get("MM_DT", "bf16")
MMDT = {"bf16": mybir.dt.bfloat16, "f32": F32, "f32r": R32}[MM_DT]
NPMM = None  # numpy dtype for host-side arrays feeding matmuls; set below


def _mr(ap):
    return ap

_cache = {}


def build_nc(trace_label=""):
    nc = bacc.Bacc("TRN2", target_bir_lowering=False, debug=False, num_devices=NCORE)

    # Host-prepacked to the exact SBUF layouts: every DMA row is contiguous
    # DRAM (few large descriptors instead of thousands of 512B ones).
    xT_d = nc.dram_tensor("xT", [128, NCH * KT * CH], MMDT,
                          kind="ExternalInput").ap()
    wq_d = nc.dram_tensor("wq", [128, KT * HPC * HD], MMDT,
                          kind="ExternalInput").ap()
    wck_d = nc.dram_tensor("wck", [128, KT * HD], MMDT,
                           kind="ExternalInput").ap()
    wcv_d = nc.dram_tensor("wcv", [128, KT * HD], MMDT,
                           kind="ExternalInput").ap()
    wo_d = nc.dram_tensor("wo", [128, HPC * D], MMDT, kind="ExternalInput").ap()
    cos2_d = nc.dram_tensor("cos2", [HD, T], F32, kind="ExternalInput").ap()
    sin2_d = nc.dram_tensor("sin2", [HD, T], F32, kind="ExternalInput").ap()
    tri_lo_d = nc.dram_tensor("tri_lo", [128, 128], F32, kind="ExternalInput").ap()
    tri_hi_d = nc.dram_tensor("tri_hi", [128, 128], F32, kind="ExternalInput").ap()
    eye_d = nc.dram_tensor("eye", [128, 128], F32, kind="ExternalInput").ap()
    jt_d = nc.dram_tensor("jt", [128, 128], MMDT, kind="ExternalInput").ap()
    bias3_d = nc.dram_tensor("bias3", [HD, 3], F32, kind="ExternalInput").ap()
    out_d = nc.dram_tensor("out", [T, D], mybir.dt.bfloat16,
                           kind="ExternalOutput").ap()

    with tile.TileContext(nc) as tc:
        with tc.tile_pool(name="persist", bufs=1) as pp:
            wq_sb = pp.tile([128, KT * HPC * HD], MMDT, tag="wq")
            wck_sb = pp.tile([128, KT * HD], MMDT, tag="wck")
            wcv_sb = pp.tile([128, KT * HD], MMDT, tag="wcv")
            wo_sb = pp.tile([128, HPC * D], MMDT, tag="wo")
            cos2_sb = pp.tile([128, T], F32, tag="cos2")
            sin2_sb = pp.tile([128, T], F32, tag="sin2")
            tri_lo = pp.tile([128, 128], F32, tag="tri_lo")
            tri_hi = pp.tile([128, 128], F32, tag="tri_hi")
            eye_sb = pp.tile([128, 128], F32, tag="eye")
            jt_sb = pp.tile([128, 128], MMDT, tag="jt")
            eyeb = pp.tile([128, 128], MMDT, tag="eyeb")
            bias3 = pp.tile([128, 3], F32, tag="bias3")
            qT_sb = pp.tile([128, HPC * T], MMDT, tag="qT")
            kT_sb = pp.tile([128, T], MMDT, tag="kT")
            v_sb = pp.tile([128, KT * HD], MMDT, tag="v")
            kbT = pp.tile([128, NB], MMDT, tag="kbT")

            # PE warmup: trip the HAM clock-gate to 8/8 while the initial
            # DMAs are in flight so real matmuls start at 2.4 GHz.
            warm_sb = pp.tile([128, 512], MMDT, tag="warm")
            with tc.tile_pool(name="psW", bufs=2, space="PSUM") as psW:
                NWARM = 20
                for i in range(NWARM):
                    wps = psW.tile([128, 512], F32, tag="w", name=f"warm{i}")
                    nc.tensor.matmul(wps[:], lhsT=warm_sb[:, :128],
                                     rhs=warm_sb[:], start=True, stop=True)
                    if i == NWARM - 1:
                        nc.vector.tensor_copy(warm_sb[:, :512], wps[:])

            # DMA triggers cost ~650ns each and serialize on the issuing
            # engine's queue; spread them across idle engines at startup.
            def dma_split(eng, dst, src, ncols, npiece):
                w = ncols // npiece
                for i in range(npiece):
                    eng.dma_start(dst[:, i * w:(i + 1) * w],
                                  src[:, i * w:(i + 1) * w])

            # ---------- phase 1: kT/vT first, then qT ----------
            # xT stays resident in SBUF (bf16: 64KB/partition), ch-major.
            xt_sb = pp.tile([128, NCH * KT * CH], MMDT, tag="xt")
            vT_sb = pp.tile([128, T], MMDT, tag="vT")
            boost_all = pp.tile([128, QT * HPC * NB], F32, tag="boost_all")
            # k/v weights + x chunk 0 gate the first matmuls
            dma_split(nc.gpsimd, wck_sb, wck_d, KT * HD, 2)
            dma_split(nc.gpsimd, wcv_sb, wcv_d, KT * HD, 2)
            dma_split(nc.sync, xt_sb, xT_d, NCH * KT * CH, 16)
            nc.gpsimd.dma_start(bias3[:], bias3_d)
            dma_split(nc.gpsimd, cos2_sb, cos2_d, T, 2)
            dma_split(nc.gpsimd, sin2_sb, sin2_d, T, 2)
            nc.gpsimd.dma_start(jt_sb[:], jt_d)
            nc.scalar.dma_start(tri_lo[:], tri_lo_d)
            nc.scalar.dma_start(tri_hi[:], tri_hi_d)
            nc.scalar.dma_start(eye_sb[:], eye_d)
            nc.vector.tensor_copy(eyeb[:], eye_sb[:])
            dma_split(nc.scalar, wq_sb, wq_d, KT * HPC * HD, 4)
            dma_split(nc.scalar, wo_sb, wo_d, HPC * D, 2)
            with tc.tile_pool(name="rs", bufs=3) as rsp, \
                 tc.tile_pool(name="psA", bufs=4, space="PSUM") as psA, \
                 tc.tile_pool(name="psR", bufs=2, space="PSUM") as psR, \
                 tc.tile_pool(name="psB", bufs=2, space="PSUM") as psB, \
                 tc.tile_pool(name="pTk", bufs=6) as pTk:
                def rope_one(ps, dst, ti, cs, nm):
                    # rope + bias: dst = (ps+b)*cos2 + J @ ((ps+b)*sin2)
                    # (J = [[0,-I64],[I64,0]] does the half-swap on the PE;
                    #  cross-partition DVE ops are illegal on HW)
                    U = rsp.tile([128, CH], F32, tag="U", name=f"U{nm}")
                    Wt = rsp.tile([128, CH], MMDT, tag="W", name=f"Wt{nm}")
                    b = bias3[:, ti:ti + 1]
                    nc.vector.scalar_tensor_tensor(
                        U[:], ps[:], b, cos2_sb[:, cs], op0=OP.add, op1=OP.mult)
                    nc.vector.scalar_tensor_tensor(
                        Wt[:], ps[:], b, sin2_sb[:, cs], op0=OP.add, op1=OP.mult)
                    rp = psR.tile([128, CH], F32, tag="rp", name=f"rp{nm}")
                    nc.tensor.matmul(rp[:], lhsT=jt_sb[:], rhs=Wt[:],
                                     start=True, stop=True)
                    nc.vector.tensor_add(dst, rp[:], U[:])

                def kv_mms(ch):
                    kTp = psA.tile([128, CH], F32, tag="qkT", name=f"kTp{ch}")
                    vTp = psA.tile([128, CH], F32, tag="qkT", name=f"vTp{ch}")
                    for kt in range(KT):
                        xt = xt_sb[:, (ch * KT + kt) * CH:(ch * KT + kt + 1) * CH]
                        st = dict(start=(kt == 0), stop=(kt == KT - 1))
                        nc.tensor.matmul(
                            kTp[:], lhsT=wck_sb[:, kt * HD:(kt + 1) * HD],
                            rhs=xt, **st)
                        nc.tensor.matmul(
                            vTp[:], lhsT=wcv_sb[:, kt * HD:(kt + 1) * HD],
                            rhs=xt, **st)
                    return kTp, vTp

                def kv_post(ch, kTp, vTp):
                    cs = slice(ch * CH, (ch + 1) * CH)
                    rope_one(kTp, kT_sb[:, cs], 2, cs, f"k{ch}")
                    nc.any.tensor_copy(vT_sb[:, cs], vTp[:])
                    # block sums of this chunk of roped kT (means after x1/BS)
                    nb_c = CH // BS
                    with nc.allow_low_precision(reason="bf16 block-mean out"):
                        nc.vector.reduce_sum(
                            kbT[:, ch * nb_c:(ch + 1) * nb_c, None],
                            kT_sb[:, cs].rearrange("p (b i) -> p b i", b=nb_c),
                            axis=mybir.AxisListType.X)

                def q_mms(ch):
                    qd = [psA.tile([128, CH], F32, tag="qkT", name=f"qd{ch}_{_h}")
                          for _h in range(HPC)]
                    for kt in range(KT):
                        xt = xt_sb[:, (ch * KT + kt) * CH:(ch * KT + kt + 1) * CH]
                        st = dict(start=(kt == 0), stop=(kt == KT - 1))
                        for h in range(HPC):
                            nc.tensor.matmul(
                                qd[h][:],
                                lhsT=wq_sb[:, kt * HPC * HD + h * HD:
                                           kt * HPC * HD + (h + 1) * HD],
                                rhs=xt, **st)
                    return qd

                def q_post(ch, qd):
                    cs = slice(ch * CH, (ch + 1) * CH)
                    for h in range(HPC):
                        rope_one(qd[h], qT_sb[:, h * T + ch * CH:
                                               h * T + (ch + 1) * CH],
                                 h, cs, f"q{ch}_{h}")
                    # block scores + top-16 boost for this chunk's q-tiles
                    for qt in range(4 * ch, 4 * ch + 4):
                        for h in range(HPC):
                            qTh = qT_sb[:, h * T + qt * 128:h * T + (qt + 1) * 128]
                            bsc = psB.tile([128, NB], F32, tag="bsc",
                                           name=f"bsc{qt}_{h}")
                            nc.tensor.matmul(bsc[:], lhsT=qTh, rhs=kbT[:],
                                             start=True, stop=True)
                            z = pTk.tile([128, NB], F32, tag="z",
                                         name=f"z{qt}_{h}")
                            z2 = pTk.tile([128, NB], F32, tag="z2",
                                          name=f"z2_{qt}_{h}")
                            m8 = pTk.tile([128, 8], F32, tag="m8",
                                          name=f"m8_{qt}_{h}")
                            t16 = pTk.tile([128, 1], F32, tag="t16",
                                           name=f"t16_{qt}_{h}")
                            nc.scalar.copy(z[:], bsc[:])
                            # top-16 threshold: t16 = min(second round of max8)
                            nc.vector.max(out=m8[:], in_=z[:])
                            nc.vector.match_replace(
                                out=z2[:], in_to_replace=m8[:], in_values=z[:],
                                imm_value=NEG)
                            nc.vector.max(out=m8[:], in_=z2[:])
                            nc.vector.tensor_reduce(
                                t16[:], m8[:], axis=mybir.AxisListType.X,
                                op=OP.min)
                            bo_sl = boost_all[:, (h * QT + qt) * NB:
                                              (h * QT + qt + 1) * NB]
                            nc.vector.scalar_tensor_tensor(
                                bo_sl, z[:], t16[:], z[:],
                                op0=OP.is_ge, op1=OP.mult)

                prev = None
                for ch in range(NCH):
                    cur = kv_mms(ch)
                    if prev is not None:
                        kv_post(ch - 1, *prev)
                    prev = cur
                kv_post(NCH - 1, *prev)
                nc.vector.tensor_scalar_mul(kbT[:], kbT[:], 1.0 / BS)

                # v[t, d] from vT[d, t] via one xbar transpose (bf16)
                nc.sync.dma_start_transpose(
                    v_sb[:].rearrange("p (k f) -> p k f", k=KT), vT_sb[:])

                prevq = None
                for ch in range(NCH):
                    curq = q_mms(ch)
                    if prevq is not None:
                        q_post(ch - 1, prevq)
                    prevq = curq
                q_post(NCH - 1, prevq)

            # ---------- phase 2: attention + projection ----------
            # Stage A(qt): S matmuls -> fused boost-add copy to SBUF (frees
            # PSUM) -> masks -> exp -> normalize to bf16.
            # Stage B(qt): xbar-transpose P -> AV -> projection -> DMA out.
            # Emitted software-pipelined: A(qt) then B(qt-1), so PE always has
            # adjacent work and HAM stays warm.
            with tc.tile_pool(name="psS", bufs=2, space="PSUM") as psS, \
                 tc.tile_pool(name="psAcc", bufs=2, space="PSUM") as psAcc, \
                 tc.tile_pool(name="psPT", bufs=2, space="PSUM") as psPT, \
                 tc.tile_pool(name="pSb", bufs=6) as pSb, \
                 tc.tile_pool(name="pPn", bufs=3) as pPn, \
                 tc.tile_pool(name="pPT", bufs=2) as pPT, \
                 tc.tile_pool(name="pA", bufs=4) as pA, \
                 tc.tile_pool(name="pOut", bufs=2) as pOut, \
                 tc.tile_pool(name="pSm", bufs=8) as pSm:
                pn_tiles = {}

                def stage_a(qt):
                    nk = min(qt, 4) + 1
                    kt0 = qt + 1 - nk
                    Pn = pPn.tile([128, 2 * 640], MMDT, tag="Pn", name=f"Pn{qt}")
                    pn_tiles[qt] = Pn
                    for h in range(HPC):
                        qTh = qT_sb[:, h * T + qt * 128:h * T + (qt + 1) * 128]
                        S = psS.tile([128, 640], F32, tag="S", name=f"S{qt}_{h}")
                        o = 0
                        while o < nk * 128:          # <=512-wide, bank-aligned
                            w = min(512, nk * 128 - o)
                            nc.tensor.matmul(
                                S[:, o:o + w], lhsT=qTh,
                                rhs=kT_sb[:, kt0 * 128 + o:kt0 * 128 + o + w],
                                start=True, stop=True)
                            o += w
                        # psum->sbuf copy (frees PSUM regardless of top-k
                        # progress), then boost broadcast-add on idle GpSimd
                        Ssb = pSb.tile([128, 640], F32, tag="Ssb", name=f"Ssb{qt}_{h}")
                        nc.vector.tensor_copy(Ssb[:, :nk * 128], S[:, :nk * 128])
                        bo_sl = boost_all[:, (h * QT + qt) * NB + 2 * kt0:
                                          (h * QT + qt) * NB + 2 * (qt + 1)]
                        bv = bo_sl[:, :, None].to_broadcast([128, 2 * nk, BS])
                        Sv = Ssb[:, :nk * 128].rearrange("p (b i) -> p b i", i=BS)
                        nc.gpsimd.tensor_tensor(out=Sv, in0=Sv, in1=bv, op=OP.add)
                        # sliding-window triangle masks (in SBUF)
                        if qt >= 4:
                            nc.vector.tensor_add(Ssb[:, 0:128], Ssb[:, 0:128],
                                                 tri_hi[:])
                        nc.vector.tensor_add(
                            Ssb[:, (nk - 1) * 128:nk * 128],
                            Ssb[:, (nk - 1) * 128:nk * 128], tri_lo[:])
                        # exp (scores bounded; no row-max) + row sums
                        Praw = pSb.tile([128, 640], F32, tag="P", name=f"P{qt}_{h}")
                        r = pSm.tile([128, 1], F32, tag="r", name=f"r{qt}_{h}")
                        rinv = pSm.tile([128, 1], F32, tag="rinv",
                                        name=f"rinv{qt}_{h}")
                        nc.scalar.activation(
                            Praw[:, :nk * 128], Ssb[:, :nk * 128], AF.Exp,
                            scale=float(SCALE), accum_out=r[:])
                        nc.vector.reciprocal(rinv[:], r[:])
                        # normalize + bf16 cast on ACT
                        nc.scalar.activation(
                            Pn[:, h * nk * 128:(h + 1) * nk * 128],
                            Praw[:, :nk * 128], AF.Copy, scale=rinv[:])

                def stage_b(qt):
                    nk = min(qt, 4) + 1
                    kt0 = qt + 1 - nk
                    Pn = pn_tiles.pop(qt)
                    PnT = pPT.tile([128, 2 * 640], MMDT, tag="PnT", name=f"PnT{qt}")
                    for hj in range(2 * nk):
                        ptp = psPT.tile([128, 128], MMDT, tag="ptp",
                                        name=f"ptp{qt}_{hj}")
                        nc.tensor.transpose(
                            ptp[:], Pn[:, hj * 128:(hj + 1) * 128], eyeb[:])
                        nc.any.tensor_copy(
                            PnT[:, hj * 128:(hj + 1) * 128], ptp[:])
                    a_sb = []
                    for h in range(HPC):
                        attnT = psAcc.tile([128, 128], F32, tag="acc",
                                           name=f"attnT{qt}_{h}")
                        for j in range(nk):
                            nc.tensor.matmul(
                                attnT[:],
                                lhsT=v_sb[:, (kt0 + j) * 128:(kt0 + j + 1) * 128],
                                rhs=PnT[:, (h * nk + j) * 128:(h * nk + j + 1) * 128],
                                start=(j == 0), stop=(j == nk - 1))
                        at = pA.tile([128, 128], MMDT, tag="at", name=f"at{qt}_{h}")
                        nc.any.tensor_copy(at[:], attnT[:])
                        a_sb.append(at[:])
                    ot = pOut.tile([128, D], mybir.dt.bfloat16, tag="ot",
                                   name=f"ot{qt}")
                    for e in range(4):
                        pr = psAcc.tile([128, 512], F32, tag="acc",
                                        name=f"pr{qt}_{e}")
                        for h in range(HPC):
                            nc.tensor.matmul(
                                pr[:], lhsT=a_sb[h],
                                rhs=wo_sb[:, h * D + e * 512:h * D + (e + 1) * 512],
                                start=(h == 0), stop=(h == HPC - 1))
                        nc.any.tensor_copy(ot[:, e * 512:(e + 1) * 512], pr[:])
                    nc.sync.dma_start(out_d[qt * 128:(qt + 1) * 128, :], ot[:])

                for qt in range(QT + 1):
                    if qt < QT:
                        stage_a(qt)
                    if qt >= 1:
                        stage_b(qt - 1)
    nc.compile()
    return nc


# ---------------------------------------------------------------- host side

def _np_mm(a):
    """Convert a host array to the matmul wire dtype."""
    import concourse.mybir as _mb
    return np.ascontiguousarray(a).astype(_mb.dt.np(MMDT))


def _host_prep(x, Wc, bc, Wk, bk, Wv, bv, Wq, bq, Wo, bo, loop_idx):
    f = np.float32
    x = np.asarray(x, f).reshape(T, D)
    Wc, Wk, Wv, Wq, Wo = (np.asarray(a, f) for a in (Wc, Wk, Wv, Wq, Wo))
    bc, bk, bv, bq, bo = (np.asarray(a, f) for a in (bc, bk, bv, bq, bo))
    li = int(np.asarray(loop_idx))

    # pack [D, n] weight (D = KT*128 contraction) into SBUF layout [128, KT*n]
    def pack_w(w):
        n = w.shape[1]
        return np.ascontiguousarray(
            w.reshape(KT, 128, n).transpose(1, 0, 2).reshape(128, KT * n))

    xT = x.T  # [D, T]
    # xt_sb layout: [128, NCH, KT, CH] (chunk-major)
    xp = np.ascontiguousarray(
        xT.reshape(KT, 128, NCH, CH).transpose(1, 2, 0, 3)
        .reshape(128, NCH * KT * CH))
    Wck = Wc @ Wk
    bck = bc @ Wk + bk
    Wcv = Wc @ Wv
    bcv = bc @ Wv + bv

    pos = (np.arange(T) + li * T).astype(np.float64)
    inv = 1.0 / (10000.0 ** (np.arange(0, HD, 2).astype(np.float64) / HD))
    ang = pos[:, None] * inv[None, :]                       # [T, 64]
    cos = np.cos(ang).astype(f)
    sin = np.sin(ang).astype(f)
    cos2 = np.ascontiguousarray(np.concatenate([cos, cos], axis=1).T)  # [128, T]
    sin2 = np.ascontiguousarray(np.concatenate([sin, sin], axis=1).T)

    perm = np.concatenate([np.arange(0, HD, 2), np.arange(1, HD, 2)])

    a = np.arange(128)
    tri_lo = np.where(a[None, :] <= a[:, None], 0.0, MASKV).astype(f)  # causal
    tri_hi = np.where(a[None, :] >= a[:, None], 0.0, MASKV).astype(f)
    eye = np.eye(128, dtype=f)
    J = np.zeros((128, 128), f)
    J[np.arange(64), np.arange(64) + 64] = -1.0
    J[np.arange(64) + 64, np.arange(64)] = 1.0
    jt = np.ascontiguousarray(J.T)

    in_maps = []
    bo_eff = bo.copy()
    for c in range(NCORE):
        h0 = HPC * c
        g = h0 // (NH // NKV)
        Wq_c = Wq[:, h0 * HD:(h0 + HPC) * HD].reshape(D, HPC, HD)[:, :, perm]
        Wq_c = np.ascontiguousarray(Wq_c.reshape(D, HPC * HD))
        bq_c = bq[h0 * HD:(h0 + HPC) * HD].reshape(HPC, HD)[:, perm]
        Wck_c = np.ascontiguousarray(Wck[:, g * HD:(g + 1) * HD][:, perm])
        bck_c = bck[g * HD:(g + 1) * HD][perm]
        Wcv_c = np.ascontiguousarray(Wcv[:, g * HD:(g + 1) * HD])
        bcv_c = bcv[g * HD:(g + 1) * HD]
        Wo_c = Wo[h0 * HD:(h0 + HPC) * HD, :]
        # v-bias folded through softmax (rows sum to 1): + bcv @ Wo_head, per head
        for hh in range(HPC):
            bo_eff = bo_eff + bcv_c @ Wo_c[hh * HD:(hh + 1) * HD]
        # wo_sb layout [128, HPC*D]: wo[p, h*D+j] = Wo_c[h*HD+p, j]
        Wo_p = np.ascontiguousarray(
            Wo_c.reshape(HPC, HD, D).transpose(1, 0, 2).reshape(128, HPC * D))
        bias3 = np.stack([bq_c[0], bq_c[1], bck_c], axis=1).astype(f)  # [128, 3]
        in_maps.append({
            "xT": _np_mm(xp), "wq": _np_mm(pack_w(Wq_c)),
            "wck": _np_mm(pack_w(Wck_c)), "wcv": _np_mm(pack_w(Wcv_c)),
            "wo": _np_mm(Wo_p),
            "cos2": cos2, "sin2": sin2, "tri_lo": tri_lo, "tri_hi": tri_hi,
            "eye": eye, "jt": _np_mm(jt), "bias3": bias3,
        })
    return in_maps, bo_eff


def _maybe_install_ntff_hook():
    """This axon image lacks antenv.axon_hooks; synthesize it so
    run_bass_kernel_spmd(trace=True) can capture NTFFs. Best-effort."""
    try:
        import sys
        import types
        import antenv
        if getattr(antenv, "axon_hooks", None) is not None:
            return
        from trn_agent_boot.trn_boot import _ntff_profile_via_ctypes
        hook = _ntff_profile_via_ctypes("/opt/axon/libaxon_pjrt.so")
        mod = types.ModuleType("antenv.axon_hooks")
        mod._hook = hook
        mod.get_axon_ntff_profile_hook = lambda: mod._hook
        mod.set_axon_ntff_profile_hook = lambda h: setattr(mod, "_hook", h)
        sys.modules["antenv.axon_hooks"] = mod
        antenv.axon_hooks = mod
    except Exception as e:  # profiling is optional
        print(f"ntff hook install failed: {e}")


def kernel(**inputs) -> np.ndarray:
    in_maps, bo_eff = _host_prep(**inputs)
    if "nc" not in _cache:
        _cache["nc"] = build_nc()
    trace = bool(int(os.environ.get("KERNEL_TRACE", "0")))
    if trace:
        _maybe_install_ntff_hook()
    res = run_bass_kernel_spmd(
        _cache["nc"], in_maps, core_ids=list(range(NCORE)),
        trace=trace)
    if trace:
        _cache["last_results"] = res
    out = np.zeros((T, D), np.float64)
    for r in res.results:
        out += r["out"].astype(np.float64)
    out = (out + bo_eff.astype(np.float64)).astype(np.float32)
    return out.reshape(B, T, D)

